# revision 24
# baseline (speedup 1.0000x reference)
"""Trainium2 Bass kernel for PointCloudTeacher (2x EdgeConv with KNN graph).

Single fused launch on 8 NeuronCores. Core c handles (batch b = c//2,
row-half h = c%2); its inputs are row/column-permuted so its own 2048 rows
come first -- one SPMD program serves all cores. Block1 is replicated per
core pair (each core computes the full 4096-point cloud's block1) so block2
needs no cross-core exchange; block2 then handles only the core's own 2048
query rows. All block2 operands (x1^T in tf32, bf16 lhsT stripes, -|x|^2/2
bias row) are built on device, so there is no second launch and no host
round-trip.

Per block:
  - Coarse KNN keys via single-pass TF32 (float32r) gram matmul plus a K=1
    rank-1 pass adding -|x_m|^2/2; top-8 via DVE max8/find_index8.
  - Exact re-rank of the top-6 candidates: batched indirect gather of
    candidate rows (with a -sq/2 column) and dot against the center row
    [x_n, 1] -> exact f32 keys -> exact top-5 (slot 0 is always self).
  - y_a = (s*W_a) @ x for the full cloud; y_c = (s*W_c) @ x + b' computed
    at the epilogue (BN folded into the weights; max over k commutes with
    the monotone BN+LeakyReLU). Block1 y-matmuls are single-pass f32r
    (tf32); block2 y-matmuls are bf16 with a bf16 y_a table (output-only).
  - Epilogue: self row of y_a direct-loaded + 4 indirect gathers by the
    exact top-4 indices, max over k, add y_c, LeakyReLU via Prelu(0.2),
    and (block1) append the -|x1|^2/2 column for block2's rerank table.
"""

import numpy as np
import ml_dtypes

import concourse.bass as bass
import concourse.bacc as bacc
import concourse.mybir as mybir
from concourse import masks
from concourse.tile import TileContext
from concourse.bass_utils import run_bass_kernel_spmd

dt = mybir.dt
AF = mybir.ActivationFunctionType
OP = mybir.AluOpType

P = 128
N = 4096
HALF = 2048
B = 4
C1, O1 = 512, 864
C2, O2 = 864, 1728
K = 5
NCAND = 6            # coarse candidates kept for the exact re-rank
EPS = 1e-5
SLOPE = 0.2
XA1 = 516            # block1 rerank row: x row + -sq/2 + pad
XW = 868             # x1d row: x1 (864) + -sq/2 + pad
N_TILES = N // P     # 32
H_TILES = HALF // P  # 16

KS1 = [(0, 128), (128, 128), (256, 128), (384, 128)]            # C1 chunks
KS2 = [(i * 128, 128) for i in range(6)] + [(768, 96)]          # C2 chunks


def _tf32(a):
    a = np.ascontiguousarray(a, dtype=np.float32)
    u = a.view(np.uint32).astype(np.uint64)
    u = ((u + 0x1000 + ((u >> 13) & 1)) & 0xFFFFE000).astype(np.uint32)
    return u.view(np.float32)


def _bf16(a):
    return np.ascontiguousarray(a, dtype=np.float32).astype(ml_dtypes.bfloat16)


def _mid_bcast(ap, rep):
    """(P, F) access pattern -> (P, rep, F) with 0-stride middle dim."""
    pat = [list(ap.ap[0]), [0, rep], list(ap.ap[1])]
    return bass.AP(ap.tensor, ap.offset, pat)


def _last_bcast(ap, rep):
    """(P, F) access pattern -> (P, F, rep) with 0-stride last dim."""
    pat = [list(ap.ap[0]), list(ap.ap[1]), [0, rep]]
    return bass.AP(ap.tensor, ap.offset, pat)


# ---------------------------------------------------------------- program

def _build_fused():
    nc = bacc.Bacc("TRN2", target_bir_lowering=False)

    # ---- inputs (per-core; weights replicated)
    xTr = nc.dram_tensor("xTr", [C1 + 1, N], dt.float32r, kind="ExternalInput")
    xa1 = nc.dram_tensor("xa1", [N, XA1], dt.float32, kind="ExternalInput")
    wa1 = nc.dram_tensor("wa1", [C1, O1], dt.float32r, kind="ExternalInput")
    wc1 = nc.dram_tensor("wc1", [C1, O1], dt.float32r, kind="ExternalInput")
    b1b = nc.dram_tensor("b1b", [P, O1], dt.float32, kind="ExternalInput")
    w2a = nc.dram_tensor("w2a", [C2, O2], dt.bfloat16, kind="ExternalInput")
    w2cb = nc.dram_tensor("w2cb", [C2 + 1, O2], dt.bfloat16, kind="ExternalInput")
    onest = nc.dram_tensor("onest", [1, P], dt.float32r, kind="ExternalInput")
    onesb = nc.dram_tensor("onesb", [1, P], dt.bfloat16, kind="ExternalInput")
    io8 = nc.dram_tensor("io8", [P, 8], dt.float32, kind="ExternalInput")

    # ---- internal DRAM
    yad1 = nc.dram_tensor("yad1", [N, O1], dt.float32)
    x1d = nc.dram_tensor("x1d", [N, XW], dt.float32)
    x1bs = nc.dram_tensor("x1bs", [N_TILES, P, 7 * P], dt.bfloat16)
    yad2 = nc.dram_tensor("yad2", [N, O2], dt.bfloat16)
    xout = nc.dram_tensor("xout", [HALF, O2], dt.float32, kind="ExternalOutput")

    OS1 = [(0, 512), (512, 352)]
    OS2 = [(0, 512), (512, 352), (864, 512), (1376, 352)]

    with TileContext(nc) as tc:
        with tc.tile_pool(name="ppg", bufs=1) as ppg:
            onest_t = ppg.tile([1, P], dt.float32r, tag="onest")
            nc.sync.dma_start(onest_t[:], onest[:])
            onesb_t = ppg.tile([1, P], dt.bfloat16, tag="onesb")
            nc.sync.dma_start(onesb_t[:], onesb[:])
            io8t = ppg.tile([P, 8], dt.float32, tag="io8")
            nc.sync.dma_start(io8t[:], io8[:])

            pidx1_cm = tc.tile_pool(name="pidx1", bufs=N_TILES)
            pidx1 = pidx1_cm.__enter__()
            idx1_t = []
            pidx2_cm = tc.tile_pool(name="pidx2", bufs=H_TILES)
            pidx2 = pidx2_cm.__enter__()
            idx2_t = []

            # ---------------- shared rerank helper ----------------
            def rerank(pbs, pbc, t, keys, xa_dram, xa_w, cdata, idx_pool, idx_list):
                cs = slice(t * P, (t + 1) * P)
                top8 = pbs.tile([P, 8], dt.float32, tag="top8")
                nc.vector.max(out=top8[:], in_=keys[:])
                cidx = pbs.tile([P, 8], dt.uint32, tag="cidx")
                nc.vector.max_index(cidx[:], top8[:], keys[:])
                cidxf = pbs.tile([P, 8], dt.float32, tag="cidxf")
                nc.vector.tensor_copy(cidxf[:], cidx[:])

                cent = pbc.tile([P, xa_w], dt.float32, tag="cent")
                nc.sync.dma_start(cent[:], xa_dram[cs, :])
                ekeys = pbs.tile([P, 8], dt.float32, tag="ekeys")
                nc.vector.memset(ekeys[:, NCAND:8], -1e30)
                cand = pbc.tile([P, 3, xa_w], dt.float32, tag="cand")
                # slot 0 = self (copy of the center row, pre-memset)
                nc.scalar.activation(cand[:, 0, :], cent[:], AF.Copy)
                nc.vector.memset(cent[:, cdata:cdata + 1], 1.0)
                for (j0, jn) in ((0, 3), (3, 3)):
                    if j0 > 0:
                        cand = pbc.tile([P, 3, xa_w], dt.float32, tag="cand")
                    for j in range(jn):
                        if j0 + j == 0:
                            continue
                        nc.gpsimd.indirect_dma_start(
                            out=cand[:, j, :],
                            out_offset=None,
                            in_=xa_dram[:],
                            in_offset=bass.IndirectOffsetOnAxis(
                                ap=cidx[:, j0 + j:j0 + j + 1], axis=0
                            ),
                        )
                    nc.gpsimd.tensor_tensor(
                        out=cand[:, :jn, :], in0=cand[:, :jn, :],
                        in1=_mid_bcast(cent[:], jn), op=OP.mult,
                    )
                    nc.vector.tensor_reduce(
                        out=ekeys[:, j0:j0 + jn], in_=cand[:, :jn, :],
                        axis=mybir.AxisListType.X, op=OP.add,
                    )
                etop = pbs.tile([P, 8], dt.float32, tag="etop")
                nc.vector.max(out=etop[:], in_=ekeys[:])
                epos = pbs.tile([P, 8], dt.uint32, tag="epos")
                nc.vector.max_index(epos[:], etop[:], ekeys[:])
                eposf = pbs.tile([P, 8], dt.float32, tag="eposf")
                nc.vector.tensor_copy(eposf[:], epos[:])

                KR = K - 1
                m48 = pbs.tile([P, KR, NCAND], dt.float32, tag="m48")
                nc.vector.tensor_tensor(
                    out=m48[:], in0=_mid_bcast(io8t[:, 0:NCAND], KR),
                    in1=_last_bcast(eposf[:, 1:K], NCAND), op=OP.is_equal,
                )
                nc.vector.tensor_tensor(
                    out=m48[:], in0=m48[:],
                    in1=_mid_bcast(cidxf[:, 0:NCAND], KR), op=OP.mult,
                )
                idx4f = pbs.tile([P, KR], dt.float32, tag="idx4f")
                nc.vector.tensor_reduce(
                    out=idx4f[:], in_=m48[:], axis=mybir.AxisListType.X,
                    op=OP.add,
                )
                idx4 = idx_pool.tile([P, KR], dt.uint32, tag="idx4")
                nc.vector.tensor_copy(idx4[:], idx4f[:])
                idx_list.append(idx4)

            # ================ PHASE A: block1 gram/rerank/y_a ================
            with tc.tile_pool(name="pp1", bufs=1) as pp1:
                xtr_t = []
                for ci, (off, kk) in enumerate(KS1):
                    tt = pp1.tile([kk, N], dt.float32r, tag=f"xtr{ci}")
                    nc.sync.dma_start(tt[:], xTr[off:off + kk, :])
                    xtr_t.append(tt)
                brow1 = pp1.tile([1, N], dt.float32r, tag="brow1")
                nc.sync.dma_start(brow1[:], xTr[C1:C1 + 1, :])
                wa1_t, wc1_t = [], []
                for ci, (off, kk) in enumerate(KS1):
                    tt = pp1.tile([kk, O1], dt.float32r, tag=f"wa1{ci}")
                    nc.sync.dma_start(tt[:], wa1[off:off + kk, :])
                    wa1_t.append(tt)
                    tt = pp1.tile([kk, O1], dt.float32r, tag=f"wc1{ci}")
                    nc.sync.dma_start(tt[:], wc1[off:off + kk, :])
                    wc1_t.append(tt)
                b1t = pp1.tile([P, O1], dt.float32, tag="b1t")
                nc.sync.dma_start(b1t[:], b1b[:])

                with (
                    tc.tile_pool(name="pk", bufs=2) as pk,
                    tc.tile_pool(name="pbs", bufs=2) as pbs,
                    tc.tile_pool(name="pbc", bufs=2) as pbc,
                    tc.tile_pool(name="pac", bufs=2) as pac,
                    tc.tile_pool(name="pgq", bufs=2, space="PSUM") as pgq,
                    tc.tile_pool(name="py1", bufs=2, space="PSUM") as py1,
                ):
                    for t in range(N_TILES):
                        cs = slice(t * P, (t + 1) * P)
                        # ---- coarse gram keys in (128, 1024) psum quarters
                        keys = pk.tile([P, N], dt.float32, tag="keys")
                        for q in range(4):
                            ps = pgq.tile([P, 1024], dt.float32, tag="gps")
                            for si in range(2):
                                nsl = slice(q * 1024 + si * 512,
                                            q * 1024 + si * 512 + 512)
                                psl = slice(si * 512, si * 512 + 512)
                                for ci, (off, kk) in enumerate(KS1):
                                    nc.tensor.matmul(
                                        ps[:, psl],
                                        lhsT=xtr_t[ci][:kk, cs],
                                        rhs=xtr_t[ci][:kk, nsl],
                                        start=(ci == 0), stop=False,
                                        skip_group_check=True,
                                    )
                                nc.tensor.matmul(
                                    ps[:, psl],
                                    lhsT=onest_t[:],
                                    rhs=brow1[:, nsl],
                                    start=False, stop=True,
                                    skip_group_check=True,
                                )
                            nc.scalar.activation(
                                keys[:, q * 1024:(q + 1) * 1024], ps[:], AF.Copy
                            )
                        # ---- top-6 + exact rerank -> idx4
                        rerank(pbs, pbc, t, keys, xa1, XA1, C1, pidx1, idx1_t)
                        # ---- y1_a tile (interleaved)
                        ps = py1.tile([P, O1], dt.float32, tag="yps")
                        for (oo, ow) in OS1:
                            for ci, (off, kk) in enumerate(KS1):
                                nc.tensor.matmul(
                                    ps[:, oo:oo + ow],
                                    lhsT=xtr_t[ci][:kk, cs],
                                    rhs=wa1_t[ci][:, oo:oo + ow],
                                    start=(ci == 0), stop=(ci == len(KS1) - 1),
                                    skip_group_check=True,
                                )
                        sb = pac.tile([P, O1], dt.float32, tag="ya_sb")
                        nc.scalar.activation(sb[:], ps[:], AF.Copy)
                        nc.sync.dma_start(yad1[cs, :], sb[:])

                tc.strict_bb_all_engine_barrier()

                # ================ PHASE B: block1 epilogue ================
                with (
                    tc.tile_pool(name="pg5", bufs=2) as pg5,
                    tc.tile_pool(name="pe2", bufs=2) as pe2,
                    tc.tile_pool(name="pyc", bufs=2, space="PSUM") as pyc,
                ):
                    for t in range(N_TILES):
                        cs = slice(t * P, (t + 1) * P)
                        # y1_c for this tile (psum -> +b1 -> sbuf)
                        ps = pyc.tile([P, O1], dt.float32, tag="ycps")
                        for (oo, ow) in OS1:
                            for ci, (off, kk) in enumerate(KS1):
                                nc.tensor.matmul(
                                    ps[:, oo:oo + ow],
                                    lhsT=xtr_t[ci][:kk, cs],
                                    rhs=wc1_t[ci][:, oo:oo + ow],
                                    start=(ci == 0), stop=(ci == len(KS1) - 1),
                                    skip_group_check=True,
                                )
                        yct = pe2.tile([P, O1], dt.float32, tag="yct")
                        nc.vector.tensor_tensor(out=yct[:], in0=ps[:],
                                                in1=b1t[:], op=OP.add)
                        # gather the 5 neighbor rows of y_a
                        g5 = pg5.tile([P, K, O1], dt.float32, tag="g5")
                        nc.sync.dma_start(g5[:, 0, :], yad1[cs, :])
                        for j in range(K - 1):
                            nc.gpsimd.indirect_dma_start(
                                out=g5[:, j + 1, :],
                                out_offset=None,
                                in_=yad1[:],
                                in_offset=bass.IndirectOffsetOnAxis(
                                    ap=idx1_t[t][:, j:j + 1], axis=0
                                ),
                            )
                        mb = pe2.tile([P, O1], dt.float32, tag="mb")
                        nc.vector.tensor_tensor(out=mb[:], in0=g5[:, 0, :],
                                                in1=g5[:, 1, :], op=OP.max)
                        for j in range(2, K):
                            nc.vector.tensor_tensor(out=mb[:], in0=mb[:],
                                                    in1=g5[:, j, :], op=OP.max)
                        xx = pe2.tile([P, XW], dt.float32, tag="xx")
                        nc.vector.tensor_tensor(out=xx[:, 0:O1], in0=mb[:],
                                                in1=yct[:], op=OP.add)
                        nc.scalar.activation(xx[:, 0:O1], xx[:, 0:O1],
                                             AF.Prelu, alpha=SLOPE)
                        # -|x1|^2/2 column + zero pad
                        sq = pe2.tile([P, O1], dt.float32, tag="sqscratch")
                        nc.scalar.activation(sq[:], xx[:, 0:O1], AF.Square,
                                             accum_out=xx[:, O1:O1 + 1])
                        nc.vector.tensor_scalar_mul(xx[:, O1:O1 + 1],
                                                    xx[:, O1:O1 + 1], -0.5)
                        nc.vector.memset(xx[:, O1 + 1:XW], 0.0)
                        nc.sync.dma_start(x1d[cs, :], xx[:])

            tc.strict_bb_all_engine_barrier()

            # ================ PHASE P2: transpose x1 -> x1T + stripes =======
            with tc.tile_pool(name="pw2", bufs=1) as pw2:
                w2a_t = []
                for ci, (off, kk) in enumerate(KS2):
                    tt = pw2.tile([kk, O2], dt.bfloat16, tag=f"w2a{ci}")
                    nc.sync.dma_start(tt[:], w2a[off:off + kk, :])
                    w2a_t.append(tt)

                px1T_cm = tc.tile_pool(name="px1T", bufs=1)
                px1T = px1T_cm.__enter__()
                x1T = []
                for j in range(7):
                    rows = 128 if j < 6 else 96
                    x1T.append(px1T.tile([rows, N], dt.float32r,
                                         tag=f"x1T{j}", name=f"x1T{j}"))
                # -|x1|^2/2 keys-bias row, base partition 0 (PE operand)
                brow2 = px1T.tile([1, N], dt.float32r, tag="brow2", name="brow2")

                with (
                    tc.tile_pool(name="pld", bufs=2) as pld,
                    tc.tile_pool(name="pstb", bufs=2) as pstb,
                    tc.tile_pool(name="ptp", bufs=4, space="PSUM") as ptp,
                ):
                    ident = pld.tile([P, P], dt.float32, tag="ident")
                    masks.make_identity(nc, ident[:])
                    for t in range(N_TILES):
                        cs = slice(t * P, (t + 1) * P)
                        xx = pld.tile([P, XW], dt.float32, tag="xxl")
                        nc.sync.dma_start(xx[:], x1d[cs, :])
                        stb = pstb.tile([P, 7 * P], dt.bfloat16, tag="stb")
                        nc.gpsimd.memset(stb[96:128, 6 * P:7 * P], 0.0)
                        for j in range(7):
                            w = 128 if j < 6 else 97
                            kd = 128 if j < 6 else 96
                            ps = ptp.tile([P, P], dt.float32, tag="tps")
                            nc.tensor.matmul(
                                ps[:w, :], lhsT=xx[:, j * P:j * P + w],
                                rhs=ident[:], is_transpose=True,
                                start=True, stop=True, skip_group_check=True,
                            )
                            nc.scalar.activation(
                                x1T[j][:kd, cs], ps[:kd, :], AF.Copy
                            )
                            if j == 6:
                                nc.scalar.activation(
                                    brow2[:, cs], ps[96:97, :], AF.Copy
                                )
                            nc.vector.tensor_copy(
                                stb[:kd, j * P:(j + 1) * P], ps[:kd, :]
                            )
                        nc.sync.dma_start(x1bs[t], stb[:])

                tc.strict_bb_all_engine_barrier()

                # ================ PHASE C: block2 gram/rerank/y_a ============
                with (
                    tc.tile_pool(name="pk2", bufs=1) as pk2,
                    tc.tile_pool(name="pbs2", bufs=2) as pbs2,
                    tc.tile_pool(name="pbc2", bufs=1) as pbc2,
                    tc.tile_pool(name="pstr", bufs=3) as pstr,
                    tc.tile_pool(name="pac2", bufs=2) as pac2,
                    tc.tile_pool(name="pgq2", bufs=2, space="PSUM") as pgq2,
                    tc.tile_pool(name="py2", bufs=2, space="PSUM") as py2,
                ):
                    def y2a_tile(yt):
                        cs = slice(yt * P, (yt + 1) * P)
                        stq = pstr.tile([P, 7 * P], dt.bfloat16, tag="stq")
                        nc.sync.dma_start(stq[:], x1bs[yt])
                        sb = pac2.tile([P, O2], dt.bfloat16, tag="y2a_sb")
                        for hh in range(2):
                            ps = py2.tile([P, 864], dt.float32, tag="y2ps",
                                          name=f"y2ps{hh}")
                            for (oo, ow) in OS2[hh * 2:hh * 2 + 2]:
                                po = oo - hh * 864
                                for ci, (off, kk) in enumerate(KS2):
                                    nc.tensor.matmul(
                                        ps[:, po:po + ow],
                                        lhsT=stq[:kk, ci * P:(ci + 1) * P],
                                        rhs=w2a_t[ci][:, oo:oo + ow],
                                        start=(ci == 0), stop=(ci == len(KS2) - 1),
                                        skip_group_check=True,
                                    )
                            nc.scalar.activation(
                                sb[:, hh * 864:(hh + 1) * 864], ps[:], AF.Copy
                            )
                        nc.sync.dma_start(yad2[cs, :], sb[:])

                    for t in range(H_TILES):
                        cs = slice(t * P, (t + 1) * P)
                        keys = pk2.tile([P, N], dt.float32, tag="keys2")
                        for q in range(4):
                            ps = pgq2.tile([P, 1024], dt.float32, tag="gps2")
                            for si in range(2):
                                nsl = slice(q * 1024 + si * 512,
                                            q * 1024 + si * 512 + 512)
                                psl = slice(si * 512, si * 512 + 512)
                                for ci, (off, kk) in enumerate(KS2):
                                    kd = 128 if ci < 6 else 96
                                    nc.tensor.matmul(
                                        ps[:, psl],
                                        lhsT=x1T[ci][:kd, cs],
                                        rhs=x1T[ci][:kd, nsl],
                                        start=(ci == 0), stop=False,
                                        skip_group_check=True,
                                    )
                                nc.tensor.matmul(
                                    ps[:, psl],
                                    lhsT=onest_t[:],
                                    rhs=brow2[:, nsl],
                                    start=False, stop=True,
                                    skip_group_check=True,
                                )
                            nc.scalar.activation(
                                keys[:, q * 1024:(q + 1) * 1024], ps[:], AF.Copy
                            )
                        rerank(pbs2, pbc2, t, keys, x1d, XW, C2, pidx2, idx2_t)
                        y2a_tile(2 * t)
                        y2a_tile(2 * t + 1)

                px1T_cm.__exit__(None, None, None)
                tc.strict_bb_all_engine_barrier()

                # ================ PHASE D: block2 epilogue ===================
                with (
                    tc.tile_pool(name="pwc2", bufs=1) as pwc2,
                    tc.tile_pool(name="pstr2", bufs=2) as pstr2,
                    tc.tile_pool(name="pg5b", bufs=2) as pg5b,
                    tc.tile_pool(name="pe2b", bufs=2) as pe2b,
                    tc.tile_pool(name="pyc2", bufs=2, space="PSUM") as pyc2,
                ):
                    w2c_t = []
                    for ci, (off, kk) in enumerate(KS2):
                        tt = pwc2.tile([kk, O2], dt.bfloat16, tag=f"w2c{ci}")
                        nc.sync.dma_start(tt[:], w2cb[off:off + kk, :])
                        w2c_t.append(tt)
                    b2row = pwc2.tile([1, O2], dt.bfloat16, tag="b2row")
                    nc.sync.dma_start(b2row[:], w2cb[C2:C2 + 1, :])
                    for t in range(H_TILES):
                        cs = slice(t * P, (t + 1) * P)
                        stq = pstr2.tile([P, 7 * P], dt.bfloat16, tag="stq2")
                        nc.sync.dma_start(stq[:], x1bs[t])
                        yct = pe2b.tile([P, O2], dt.float32, tag="yct2")
                        for hh in range(2):
                            ps = pyc2.tile([P, 864], dt.float32, tag="yc2ps",
                                           name=f"yc2ps{hh}")
                            for (oo, ow) in OS2[hh * 2:hh * 2 + 2]:
                                po = oo - hh * 864
                                for ci, (off, kk) in enumerate(KS2):
                                    nc.tensor.matmul(
                                        ps[:, po:po + ow],
                                        lhsT=stq[:kk, ci * P:(ci + 1) * P],
                                        rhs=w2c_t[ci][:, oo:oo + ow],
                                        start=(ci == 0), stop=False,
                                        skip_group_check=True,
                                    )
                                nc.tensor.matmul(
                                    ps[:, po:po + ow],
                                    lhsT=onesb_t[:],
                                    rhs=b2row[:, oo:oo + ow],
                                    start=False, stop=True,
                                    skip_group_check=True,
                                )
                            nc.scalar.activation(
                                yct[:, hh * 864:(hh + 1) * 864], ps[:], AF.Copy
                            )
                        g5 = pg5b.tile([P, K, O2], dt.bfloat16, tag="g5b")
                        nc.sync.dma_start(g5[:, 0, :], yad2[cs, :])
                        for j in range(K - 1):
                            nc.gpsimd.indirect_dma_start(
                                out=g5[:, j + 1, :],
                                out_offset=None,
                                in_=yad2[:],
                                in_offset=bass.IndirectOffsetOnAxis(
                                    ap=idx2_t[t][:, j:j + 1], axis=0
                                ),
                            )
                        mb = pe2b.tile([P, O2], dt.bfloat16, tag="mb2")
                        nc.vector.tensor_tensor(out=mb[:], in0=g5[:, 0, :],
                                                in1=g5[:, 1, :], op=OP.max)
                        for j in range(2, K):
                            nc.vector.tensor_tensor(out=mb[:], in0=mb[:],
                                                    in1=g5[:, j, :], op=OP.max)
                        xo = pe2b.tile([P, O2], dt.float32, tag="xo")
                        nc.vector.tensor_tensor(out=xo[:], in0=mb[:],
                                                in1=yct[:], op=OP.add)
                        nc.scalar.activation(xo[:], xo[:], AF.Prelu, alpha=SLOPE)
                        nc.sync.dma_start(xout[cs, :], xo[:])

            pidx2_cm.__exit__(None, None, None)
            pidx1_cm.__exit__(None, None, None)

    nc.finalize()
    return nc


_CACHE = {}


def _get_programs():
    if "p" not in _CACHE:
        _CACHE["p"] = _build_fused()
    return (_CACHE["p"],)


# ---------------------------------------------------------------- host side

def _fold_bn(W, gamma, beta, mean, var, cin):
    s = gamma.astype(np.float64) / np.sqrt(var.astype(np.float64) + EPS)
    Wp = s[:, None] * W.astype(np.float64)
    Wa = Wp[:, :cin].T
    Wc = (Wp[:, cin:] - Wp[:, :cin]).T
    bp = beta.astype(np.float64) - s * mean.astype(np.float64)
    return (np.ascontiguousarray(Wa, np.float32),
            np.ascontiguousarray(Wc, np.float32),
            bp.astype(np.float32))


def _prep_core(xp, W1a, W1c, b1, w2a_b, w2cb_b):
    xT = np.ascontiguousarray(xp.T)
    sq = np.einsum("nc,nc->n", xp.astype(np.float64), xp.astype(np.float64))
    bias_row = _tf32((-sq / 2).astype(np.float32))[None, :]
    xa = np.zeros((N, XA1), np.float32)
    xa[:, :C1] = xp
    xa[:, C1] = (-sq / 2).astype(np.float32)
    return dict(
        xTr=np.concatenate([_tf32(xT), bias_row], axis=0),
        xa1=xa,
        wa1=W1a, wc1=W1c,
        b1b=np.broadcast_to(b1, (P, O1)).copy(),
        w2a=w2a_b, w2cb=w2cb_b,
        onest=np.ones((1, P), np.float32),
        onesb=np.ones((1, P), ml_dtypes.bfloat16),
        io8=np.broadcast_to(np.arange(8, dtype=np.float32), (P, 8)).copy(),
    )


_LAST_EXEC_NS = {"l1": None}


def kernel(interm_repr, W1, bn1_gamma, bn1_beta, bn1_mean, bn1_var,
           W2, bn2_gamma, bn2_beta, bn2_mean, bn2_var, _trace=False):
    x = np.asarray(interm_repr, dtype=np.float32)
    (p,) = _get_programs()

    W1a, W1c, b1 = _fold_bn(np.asarray(W1), np.asarray(bn1_gamma),
                            np.asarray(bn1_beta), np.asarray(bn1_mean),
                            np.asarray(bn1_var), C1)
    W2a, W2c, b2 = _fold_bn(np.asarray(W2), np.asarray(bn2_gamma),
                            np.asarray(bn2_beta), np.asarray(bn2_mean),
                            np.asarray(bn2_var), C2)
    W1a = _tf32(W1a)
    W1c = _tf32(W1c)
    w2a_b = _bf16(W2a)
    w2cb_b = _bf16(np.concatenate([W2c, b2[None, :]], axis=0))

    in_maps = []
    for c in range(8):
        b, h = c // 2, c % 2
        perm = np.r_[h * HALF:(h + 1) * HALF, (1 - h) * HALF:(2 - h) * HALF]
        in_maps.append(_prep_core(x[b][perm], W1a, W1c, b1, w2a_b, w2cb_b))
    r = run_bass_kernel_spmd(p, in_maps, core_ids=list(range(8)), trace=_trace)
    _LAST_EXEC_NS["l1"] = r.exec_time_ns

    x2 = np.empty((B, N, O2), np.float32)
    for c in range(8):
        b, h = c // 2, c % 2
        x2[b, h * HALF:(h + 1) * HALF] = r.results[c]["xout"]
    return x2


if __name__ == "__main__":
    rng = np.random.default_rng(0)
    inp = dict(
        interm_repr=rng.standard_normal((B, N, C1), dtype=np.float32),
        W1=(rng.standard_normal((O1, 2 * C1)) / np.sqrt(2 * C1)).astype(np.float32),
        bn1_gamma=1 + 0.1 * rng.standard_normal(O1).astype(np.float32),
        bn1_beta=0.1 * rng.standard_normal(O1).astype(np.float32),
        bn1_mean=0.1 * rng.standard_normal(O1).astype(np.float32),
        bn1_var=0.5 + rng.random(O1).astype(np.float32),
        W2=(rng.standard_normal((O2, 2 * C2)) / np.sqrt(2 * C2)).astype(np.float32),
        bn2_gamma=1 + 0.1 * rng.standard_normal(O2).astype(np.float32),
        bn2_beta=0.1 * rng.standard_normal(O2).astype(np.float32),
        bn2_mean=0.1 * rng.standard_normal(O2).astype(np.float32),
        bn2_var=0.5 + rng.random(O2).astype(np.float32),
    )
    out = kernel(**inp)
    print("kernel out", out.shape, out.dtype, np.abs(out).mean())


# revision 30
# speedup vs baseline: 1.0103x; 1.0103x over previous
"""Trainium2 Bass kernel for PointCloudTeacher (2x EdgeConv with KNN graph).

Single fused launch on 8 NeuronCores. Core c handles (batch b = c//2,
row-half h = c%2); its inputs are row/column-permuted so its own 2048 rows
come first -- one SPMD program serves all cores. Block1 is replicated per
core pair (each core computes the full 4096-point cloud's block1) so block2
needs no cross-core exchange; block2 then handles only the core's own 2048
query rows. All block2 operands (x1^T in tf32, bf16 lhsT stripes, -|x|^2/2
bias row) are built on device, so there is no second launch and no host
round-trip.

Per block:
  - Coarse KNN keys via single-pass TF32 (float32r) gram matmul plus a K=1
    rank-1 pass adding -|x_m|^2/2; top-8 via DVE max8/find_index8.
  - Exact re-rank of the top-6 candidates: batched indirect gather of
    candidate rows (with a -sq/2 column) and dot against the center row
    [x_n, 1] -> exact f32 keys -> exact top-5 (slot 0 is always self).
  - y_a = (s*W_a) @ x for the full cloud; y_c = (s*W_c) @ x + b' computed
    at the epilogue (BN folded into the weights; max over k commutes with
    the monotone BN+LeakyReLU). Block1 y-matmuls are single-pass f32r
    (tf32); block2 y-matmuls are bf16 with a bf16 y_a table (output-only).
  - Epilogue: self row of y_a direct-loaded + 4 indirect gathers by the
    exact top-4 indices, max over k, add y_c, LeakyReLU via Prelu(0.2),
    and (block1) append the -|x1|^2/2 column for block2's rerank table.
"""

import numpy as np
import ml_dtypes

import concourse.bass as bass
import concourse.bacc as bacc
import concourse.mybir as mybir
from concourse import masks
from concourse.tile import TileContext
from concourse.bass_utils import run_bass_kernel_spmd

dt = mybir.dt
AF = mybir.ActivationFunctionType
OP = mybir.AluOpType

P = 128
N = 4096
HALF = 2048
B = 4
C1, O1 = 512, 864
C2, O2 = 864, 1728
K = 5
NCAND = 6            # coarse candidates kept for the exact re-rank
EPS = 1e-5
SLOPE = 0.2
XA1 = 516            # block1 rerank row: x row + -sq/2 + pad
XW = 868             # x1d row: x1 (864) + -sq/2 + pad
N_TILES = N // P     # 32
H_TILES = HALF // P  # 16

KS1 = [(0, 128), (128, 128), (256, 128), (384, 128)]            # C1 chunks
KS2 = [(i * 128, 128) for i in range(6)] + [(768, 96)]          # C2 chunks


def _tf32(a):
    a = np.ascontiguousarray(a, dtype=np.float32)
    u = a.view(np.uint32).astype(np.uint64)
    u = ((u + 0x1000 + ((u >> 13) & 1)) & 0xFFFFE000).astype(np.uint32)
    return u.view(np.float32)


def _bf16(a):
    return np.ascontiguousarray(a, dtype=np.float32).astype(ml_dtypes.bfloat16)


def _mid_bcast(ap, rep):
    """(P, F) access pattern -> (P, rep, F) with 0-stride middle dim."""
    pat = [list(ap.ap[0]), [0, rep], list(ap.ap[1])]
    return bass.AP(ap.tensor, ap.offset, pat)


def _last_bcast(ap, rep):
    """(P, F) access pattern -> (P, F, rep) with 0-stride last dim."""
    pat = [list(ap.ap[0]), list(ap.ap[1]), [0, rep]]
    return bass.AP(ap.tensor, ap.offset, pat)


# ---------------------------------------------------------------- program

def _build_fused():
    nc = bacc.Bacc("TRN2", target_bir_lowering=False)

    # ---- inputs (per-core; weights replicated)
    xTr = nc.dram_tensor("xTr", [C1 + 1, N], dt.float32r, kind="ExternalInput")
    xa1 = nc.dram_tensor("xa1", [N, XA1], dt.float32, kind="ExternalInput")
    wa1 = nc.dram_tensor("wa1", [C1, O1], dt.float32r, kind="ExternalInput")
    wc1 = nc.dram_tensor("wc1", [C1, O1], dt.float32r, kind="ExternalInput")
    b1b = nc.dram_tensor("b1b", [P, O1], dt.float32, kind="ExternalInput")
    w2a = nc.dram_tensor("w2a", [C2, O2], dt.bfloat16, kind="ExternalInput")
    w2cb = nc.dram_tensor("w2cb", [C2 + 1, O2], dt.bfloat16, kind="ExternalInput")
    onest = nc.dram_tensor("onest", [1, P], dt.float32r, kind="ExternalInput")
    onesb = nc.dram_tensor("onesb", [1, P], dt.bfloat16, kind="ExternalInput")
    io8 = nc.dram_tensor("io8", [P, 8], dt.float32, kind="ExternalInput")

    # ---- internal DRAM
    yad1 = nc.dram_tensor("yad1", [N, O1], dt.float32)
    x1d = nc.dram_tensor("x1d", [N, XW], dt.float32)
    x1bs = nc.dram_tensor("x1bs", [N_TILES, P, 7 * P], dt.bfloat16)
    yad2 = nc.dram_tensor("yad2", [N, O2], dt.bfloat16)
    xout = nc.dram_tensor("xout", [HALF, O2], dt.float32, kind="ExternalOutput")

    OS1 = [(0, 512), (512, 352)]
    OS2 = [(0, 512), (512, 352), (864, 512), (1376, 352)]

    with TileContext(nc) as tc:
        with tc.tile_pool(name="ppg", bufs=1) as ppg:
            onest_t = ppg.tile([1, P], dt.float32r, tag="onest")
            nc.sync.dma_start(onest_t[:], onest[:])
            onesb_t = ppg.tile([1, P], dt.bfloat16, tag="onesb")
            nc.sync.dma_start(onesb_t[:], onesb[:])
            io8t = ppg.tile([P, 8], dt.float32, tag="io8")
            nc.sync.dma_start(io8t[:], io8[:])

            pidx1_cm = tc.tile_pool(name="pidx1", bufs=N_TILES)
            pidx1 = pidx1_cm.__enter__()
            idx1_t = []
            pidx2_cm = tc.tile_pool(name="pidx2", bufs=H_TILES)
            pidx2 = pidx2_cm.__enter__()
            idx2_t = []

            # ---------------- shared rerank helper ----------------
            def rerank(pbs, pbc, t, keys, xa_dram, xa_w, cdata, idx_pool, idx_list):
                cs = slice(t * P, (t + 1) * P)
                top8 = pbs.tile([P, 8], dt.float32, tag="top8")
                nc.vector.max(out=top8[:], in_=keys[:])
                cidx = pbs.tile([P, 8], dt.uint32, tag="cidx")
                nc.vector.max_index(cidx[:], top8[:], keys[:])
                cidxf = pbs.tile([P, 8], dt.float32, tag="cidxf")
                nc.vector.tensor_copy(cidxf[:], cidx[:])

                cent = pbc.tile([P, xa_w], dt.float32, tag="cent")
                nc.sync.dma_start(cent[:], xa_dram[cs, :])
                ekeys = pbs.tile([P, 8], dt.float32, tag="ekeys")
                nc.vector.memset(ekeys[:, NCAND:8], -1e30)
                cand = pbc.tile([P, 3, xa_w], dt.float32, tag="cand")
                # slot 0 = self (copy of the center row, pre-memset)
                nc.scalar.activation(cand[:, 0, :], cent[:], AF.Copy)
                nc.vector.memset(cent[:, cdata:cdata + 1], 1.0)
                for (j0, jn) in ((0, 3), (3, 3)):
                    if j0 > 0:
                        cand = pbc.tile([P, 3, xa_w], dt.float32, tag="cand")
                    for j in range(jn):
                        if j0 + j == 0:
                            continue
                        nc.gpsimd.indirect_dma_start(
                            out=cand[:, j, :],
                            out_offset=None,
                            in_=xa_dram[:],
                            in_offset=bass.IndirectOffsetOnAxis(
                                ap=cidx[:, j0 + j:j0 + j + 1], axis=0
                            ),
                        )
                    nc.gpsimd.tensor_tensor(
                        out=cand[:, :jn, :], in0=cand[:, :jn, :],
                        in1=_mid_bcast(cent[:], jn), op=OP.mult,
                    )
                    nc.vector.tensor_reduce(
                        out=ekeys[:, j0:j0 + jn], in_=cand[:, :jn, :],
                        axis=mybir.AxisListType.X, op=OP.add,
                    )
                etop = pbs.tile([P, 8], dt.float32, tag="etop")
                nc.vector.max(out=etop[:], in_=ekeys[:])
                epos = pbs.tile([P, 8], dt.uint32, tag="epos")
                nc.vector.max_index(epos[:], etop[:], ekeys[:])
                eposf = pbs.tile([P, 8], dt.float32, tag="eposf")
                nc.vector.tensor_copy(eposf[:], epos[:])

                KR = K - 1
                m48 = pbs.tile([P, KR, NCAND], dt.float32, tag="m48")
                nc.vector.tensor_tensor(
                    out=m48[:], in0=_mid_bcast(io8t[:, 0:NCAND], KR),
                    in1=_last_bcast(eposf[:, 1:K], NCAND), op=OP.is_equal,
                )
                nc.vector.tensor_tensor(
                    out=m48[:], in0=m48[:],
                    in1=_mid_bcast(cidxf[:, 0:NCAND], KR), op=OP.mult,
                )
                idx4f = pbs.tile([P, KR], dt.float32, tag="idx4f")
                nc.vector.tensor_reduce(
                    out=idx4f[:], in_=m48[:], axis=mybir.AxisListType.X,
                    op=OP.add,
                )
                idx4 = idx_pool.tile([P, KR], dt.uint32, tag="idx4")
                nc.vector.tensor_copy(idx4[:], idx4f[:])
                idx_list.append(idx4)

            # ================ PHASE A: block1 gram/rerank/y_a ================
            with tc.tile_pool(name="pp1", bufs=1) as pp1:
                xtr_t = []
                for ci, (off, kk) in enumerate(KS1):
                    tt = pp1.tile([kk, N], dt.float32r, tag=f"xtr{ci}")
                    nc.sync.dma_start(tt[:], xTr[off:off + kk, :])
                    xtr_t.append(tt)
                brow1 = pp1.tile([1, N], dt.float32r, tag="brow1")
                nc.sync.dma_start(brow1[:], xTr[C1:C1 + 1, :])
                wa1_t, wc1_t = [], []
                for ci, (off, kk) in enumerate(KS1):
                    tt = pp1.tile([kk, O1], dt.float32r, tag=f"wa1{ci}")
                    nc.sync.dma_start(tt[:], wa1[off:off + kk, :])
                    wa1_t.append(tt)
                    tt = pp1.tile([kk, O1], dt.float32r, tag=f"wc1{ci}")
                    nc.sync.dma_start(tt[:], wc1[off:off + kk, :])
                    wc1_t.append(tt)
                b1t = pp1.tile([P, O1], dt.float32, tag="b1t")
                nc.sync.dma_start(b1t[:], b1b[:])

                with (
                    tc.tile_pool(name="pk", bufs=2) as pk,
                    tc.tile_pool(name="pbs", bufs=2) as pbs,
                    tc.tile_pool(name="pbc", bufs=2) as pbc,
                    tc.tile_pool(name="pac", bufs=2) as pac,
                    tc.tile_pool(name="pgq", bufs=2, space="PSUM") as pgq,
                    tc.tile_pool(name="py1", bufs=2, space="PSUM") as py1,
                ):
                    for t in range(N_TILES):
                        cs = slice(t * P, (t + 1) * P)
                        # ---- coarse gram keys in (128, 1024) psum quarters
                        keys = pk.tile([P, N], dt.float32, tag="keys")
                        for q in range(4):
                            ps = pgq.tile([P, 1024], dt.float32, tag="gps")
                            for si in range(2):
                                nsl = slice(q * 1024 + si * 512,
                                            q * 1024 + si * 512 + 512)
                                psl = slice(si * 512, si * 512 + 512)
                                for ci, (off, kk) in enumerate(KS1):
                                    nc.tensor.matmul(
                                        ps[:, psl],
                                        lhsT=xtr_t[ci][:kk, cs],
                                        rhs=xtr_t[ci][:kk, nsl],
                                        start=(ci == 0), stop=False,
                                        skip_group_check=True,
                                    )
                                nc.tensor.matmul(
                                    ps[:, psl],
                                    lhsT=onest_t[:],
                                    rhs=brow1[:, nsl],
                                    start=False, stop=True,
                                    skip_group_check=True,
                                )
                            nc.scalar.activation(
                                keys[:, q * 1024:(q + 1) * 1024], ps[:], AF.Copy
                            )
                        # ---- top-6 + exact rerank -> idx4
                        rerank(pbs, pbc, t, keys, xa1, XA1, C1, pidx1, idx1_t)
                        # ---- y1_a tile (interleaved)
                        ps = py1.tile([P, O1], dt.float32, tag="yps")
                        for (oo, ow) in OS1:
                            for ci, (off, kk) in enumerate(KS1):
                                nc.tensor.matmul(
                                    ps[:, oo:oo + ow],
                                    lhsT=xtr_t[ci][:kk, cs],
                                    rhs=wa1_t[ci][:, oo:oo + ow],
                                    start=(ci == 0), stop=(ci == len(KS1) - 1),
                                    skip_group_check=True,
                                )
                        sb = pac.tile([P, O1], dt.float32, tag="ya_sb")
                        nc.scalar.activation(sb[:], ps[:], AF.Copy)
                        nc.sync.dma_start(yad1[cs, :], sb[:])

                tc.strict_bb_all_engine_barrier()

                # ================ PHASE B: block1 epilogue ================
                with (
                    tc.tile_pool(name="pg5", bufs=2) as pg5,
                    tc.tile_pool(name="pe2", bufs=2) as pe2,
                    tc.tile_pool(name="pyc", bufs=2, space="PSUM") as pyc,
                ):
                    for t in range(N_TILES):
                        cs = slice(t * P, (t + 1) * P)
                        # y1_c for this tile (psum -> +b1 -> sbuf)
                        ps = pyc.tile([P, O1], dt.float32, tag="ycps")
                        for (oo, ow) in OS1:
                            for ci, (off, kk) in enumerate(KS1):
                                nc.tensor.matmul(
                                    ps[:, oo:oo + ow],
                                    lhsT=xtr_t[ci][:kk, cs],
                                    rhs=wc1_t[ci][:, oo:oo + ow],
                                    start=(ci == 0), stop=(ci == len(KS1) - 1),
                                    skip_group_check=True,
                                )
                        yct = pe2.tile([P, O1], dt.float32, tag="yct")
                        nc.vector.tensor_tensor(out=yct[:], in0=ps[:],
                                                in1=b1t[:], op=OP.add)
                        # gather the 5 neighbor rows of y_a
                        g5 = pg5.tile([P, K, O1], dt.float32, tag="g5")
                        nc.sync.dma_start(g5[:, 0, :], yad1[cs, :])
                        for j in range(K - 1):
                            nc.gpsimd.indirect_dma_start(
                                out=g5[:, j + 1, :],
                                out_offset=None,
                                in_=yad1[:],
                                in_offset=bass.IndirectOffsetOnAxis(
                                    ap=idx1_t[t][:, j:j + 1], axis=0
                                ),
                            )
                        mb = pe2.tile([P, O1], dt.float32, tag="mb")
                        nc.vector.tensor_tensor(out=mb[:], in0=g5[:, 0, :],
                                                in1=g5[:, 1, :], op=OP.max)
                        for j in range(2, K):
                            nc.vector.tensor_tensor(out=mb[:], in0=mb[:],
                                                    in1=g5[:, j, :], op=OP.max)
                        xx = pe2.tile([P, XW], dt.float32, tag="xx")
                        nc.vector.tensor_tensor(out=xx[:, 0:O1], in0=mb[:],
                                                in1=yct[:], op=OP.add)
                        nc.scalar.activation(xx[:, 0:O1], xx[:, 0:O1],
                                             AF.Prelu, alpha=SLOPE)
                        # -|x1|^2/2 column + zero pad
                        sq = pe2.tile([P, O1], dt.float32, tag="sqscratch")
                        nc.scalar.activation(sq[:], xx[:, 0:O1], AF.Square,
                                             accum_out=xx[:, O1:O1 + 1])
                        nc.vector.tensor_scalar_mul(xx[:, O1:O1 + 1],
                                                    xx[:, O1:O1 + 1], -0.5)
                        nc.vector.memset(xx[:, O1 + 1:XW], 0.0)
                        nc.sync.dma_start(x1d[cs, :], xx[:])

            # ================ PHASE P2: transpose x1 -> x1T + stripes =======
            with tc.tile_pool(name="pw2", bufs=1) as pw2:
                w2a_t = []
                for ci, (off, kk) in enumerate(KS2):
                    tt = pw2.tile([kk, O2], dt.bfloat16, tag=f"w2a{ci}")
                    nc.sync.dma_start(tt[:], w2a[off:off + kk, :])
                    w2a_t.append(tt)
                tc.strict_bb_all_engine_barrier()

                px1T_cm = tc.tile_pool(name="px1T", bufs=1)
                px1T = px1T_cm.__enter__()
                x1T = []
                for j in range(7):
                    rows = 128 if j < 6 else 96
                    x1T.append(px1T.tile([rows, N], dt.float32r,
                                         tag=f"x1T{j}", name=f"x1T{j}"))
                # -|x1|^2/2 keys-bias row, base partition 0 (PE operand)
                brow2 = px1T.tile([1, N], dt.float32r, tag="brow2", name="brow2")

                with (
                    tc.tile_pool(name="pld", bufs=2) as pld,
                    tc.tile_pool(name="pstb", bufs=2) as pstb,
                    tc.tile_pool(name="ptp", bufs=4, space="PSUM") as ptp,
                ):
                    ident = pld.tile([P, P], dt.float32, tag="ident")
                    masks.make_identity(nc, ident[:])
                    for t in range(N_TILES):
                        cs = slice(t * P, (t + 1) * P)
                        xx = pld.tile([P, XW], dt.float32, tag="xxl")
                        nc.sync.dma_start(xx[:], x1d[cs, :])
                        stb = pstb.tile([P, 7 * P], dt.bfloat16, tag="stb")
                        nc.gpsimd.memset(stb[96:128, 6 * P:7 * P], 0.0)
                        for j in range(7):
                            w = 128 if j < 6 else 97
                            kd = 128 if j < 6 else 96
                            ps = ptp.tile([P, P], dt.float32, tag="tps")
                            nc.tensor.matmul(
                                ps[:w, :], lhsT=xx[:, j * P:j * P + w],
                                rhs=ident[:], is_transpose=True,
                                start=True, stop=True, skip_group_check=True,
                            )
                            nc.scalar.activation(
                                x1T[j][:kd, cs], ps[:kd, :], AF.Copy
                            )
                            if j == 6:
                                nc.scalar.activation(
                                    brow2[:, cs], ps[96:97, :], AF.Copy
                                )
                            nc.vector.tensor_copy(
                                stb[:kd, j * P:(j + 1) * P], ps[:kd, :]
                            )
                        nc.sync.dma_start(x1bs[t], stb[:])

                tc.strict_bb_all_engine_barrier()

                # ================ PHASE C: block2 gram/rerank/y_a ============
                with (
                    tc.tile_pool(name="pk2", bufs=1) as pk2,
                    tc.tile_pool(name="pbs2", bufs=2) as pbs2,
                    tc.tile_pool(name="pbc2", bufs=1) as pbc2,
                    tc.tile_pool(name="pstr", bufs=3) as pstr,
                    tc.tile_pool(name="pac2", bufs=2) as pac2,
                    tc.tile_pool(name="pgq2", bufs=2, space="PSUM") as pgq2,
                    tc.tile_pool(name="py2", bufs=2, space="PSUM") as py2,
                ):
                    def y2a_tile(yt):
                        cs = slice(yt * P, (yt + 1) * P)
                        stq = pstr.tile([P, 7 * P], dt.bfloat16, tag="stq")
                        nc.sync.dma_start(stq[:], x1bs[yt])
                        sb = pac2.tile([P, O2], dt.bfloat16, tag="y2a_sb")
                        for hh in range(2):
                            ps = py2.tile([P, 864], dt.float32, tag="y2ps",
                                          name=f"y2ps{hh}")
                            for (oo, ow) in OS2[hh * 2:hh * 2 + 2]:
                                po = oo - hh * 864
                                for ci, (off, kk) in enumerate(KS2):
                                    nc.tensor.matmul(
                                        ps[:, po:po + ow],
                                        lhsT=stq[:kk, ci * P:(ci + 1) * P],
                                        rhs=w2a_t[ci][:, oo:oo + ow],
                                        start=(ci == 0), stop=(ci == len(KS2) - 1),
                                        skip_group_check=True,
                                    )
                            nc.scalar.activation(
                                sb[:, hh * 864:(hh + 1) * 864], ps[:], AF.Copy
                            )
                        nc.sync.dma_start(yad2[cs, :], sb[:])

                    for t in range(H_TILES):
                        cs = slice(t * P, (t + 1) * P)
                        keys = pk2.tile([P, N], dt.float32, tag="keys2")
                        for q in range(4):
                            ps = pgq2.tile([P, 1024], dt.float32, tag="gps2")
                            for si in range(2):
                                nsl = slice(q * 1024 + si * 512,
                                            q * 1024 + si * 512 + 512)
                                psl = slice(si * 512, si * 512 + 512)
                                for ci, (off, kk) in enumerate(KS2):
                                    kd = 128 if ci < 6 else 96
                                    nc.tensor.matmul(
                                        ps[:, psl],
                                        lhsT=x1T[ci][:kd, cs],
                                        rhs=x1T[ci][:kd, nsl],
                                        start=(ci == 0), stop=False,
                                        skip_group_check=True,
                                    )
                                nc.tensor.matmul(
                                    ps[:, psl],
                                    lhsT=onest_t[:],
                                    rhs=brow2[:, nsl],
                                    start=False, stop=True,
                                    skip_group_check=True,
                                )
                            nc.scalar.activation(
                                keys[:, q * 1024:(q + 1) * 1024], ps[:], AF.Copy
                            )
                        rerank(pbs2, pbc2, t, keys, x1d, XW, C2, pidx2, idx2_t)
                        y2a_tile(2 * t)
                        y2a_tile(2 * t + 1)

                px1T_cm.__exit__(None, None, None)

                # ================ PHASE D: block2 epilogue ===================
                with (
                    tc.tile_pool(name="pwc2", bufs=1) as pwc2,
                    tc.tile_pool(name="pstr2", bufs=2) as pstr2,
                    tc.tile_pool(name="pg5b", bufs=2) as pg5b,
                    tc.tile_pool(name="pe2b", bufs=2) as pe2b,
                    tc.tile_pool(name="pyc2", bufs=2, space="PSUM") as pyc2,
                ):
                    w2c_t = []
                    for ci, (off, kk) in enumerate(KS2):
                        tt = pwc2.tile([kk, O2], dt.bfloat16, tag=f"w2c{ci}")
                        nc.sync.dma_start(tt[:], w2cb[off:off + kk, :])
                        w2c_t.append(tt)
                    b2row = pwc2.tile([1, O2], dt.bfloat16, tag="b2row")
                    nc.sync.dma_start(b2row[:], w2cb[C2:C2 + 1, :])
                    tc.strict_bb_all_engine_barrier()
                    for t in range(H_TILES):
                        cs = slice(t * P, (t + 1) * P)
                        stq = pstr2.tile([P, 7 * P], dt.bfloat16, tag="stq2")
                        nc.sync.dma_start(stq[:], x1bs[t])
                        yct = pe2b.tile([P, O2], dt.float32, tag="yct2")
                        for hh in range(2):
                            ps = pyc2.tile([P, 864], dt.float32, tag="yc2ps",
                                           name=f"yc2ps{hh}")
                            for (oo, ow) in OS2[hh * 2:hh * 2 + 2]:
                                po = oo - hh * 864
                                for ci, (off, kk) in enumerate(KS2):
                                    nc.tensor.matmul(
                                        ps[:, po:po + ow],
                                        lhsT=stq[:kk, ci * P:(ci + 1) * P],
                                        rhs=w2c_t[ci][:, oo:oo + ow],
                                        start=(ci == 0), stop=False,
                                        skip_group_check=True,
                                    )
                                nc.tensor.matmul(
                                    ps[:, po:po + ow],
                                    lhsT=onesb_t[:],
                                    rhs=b2row[:, oo:oo + ow],
                                    start=False, stop=True,
                                    skip_group_check=True,
                                )
                            nc.scalar.activation(
                                yct[:, hh * 864:(hh + 1) * 864], ps[:], AF.Copy
                            )
                        g5 = pg5b.tile([P, K, O2], dt.bfloat16, tag="g5b")
                        nc.sync.dma_start(g5[:, 0, :], yad2[cs, :])
                        for j in range(K - 1):
                            nc.gpsimd.indirect_dma_start(
                                out=g5[:, j + 1, :],
                                out_offset=None,
                                in_=yad2[:],
                                in_offset=bass.IndirectOffsetOnAxis(
                                    ap=idx2_t[t][:, j:j + 1], axis=0
                                ),
                            )
                        mb = pe2b.tile([P, O2], dt.bfloat16, tag="mb2")
                        nc.vector.tensor_tensor(out=mb[:], in0=g5[:, 0, :],
                                                in1=g5[:, 1, :], op=OP.max)
                        for j in range(2, K):
                            nc.vector.tensor_tensor(out=mb[:], in0=mb[:],
                                                    in1=g5[:, j, :], op=OP.max)
                        xo = pe2b.tile([P, O2], dt.float32, tag="xo")
                        nc.vector.tensor_tensor(out=xo[:], in0=mb[:],
                                                in1=yct[:], op=OP.add)
                        nc.scalar.activation(xo[:], xo[:], AF.Prelu, alpha=SLOPE)
                        nc.sync.dma_start(xout[cs, :], xo[:])

            pidx2_cm.__exit__(None, None, None)
            pidx1_cm.__exit__(None, None, None)

    nc.finalize()
    return nc


_CACHE = {}


def _get_programs():
    if "p" not in _CACHE:
        _CACHE["p"] = _build_fused()
    return (_CACHE["p"],)


# ---------------------------------------------------------------- host side

def _fold_bn(W, gamma, beta, mean, var, cin):
    s = gamma.astype(np.float64) / np.sqrt(var.astype(np.float64) + EPS)
    Wp = s[:, None] * W.astype(np.float64)
    Wa = Wp[:, :cin].T
    Wc = (Wp[:, cin:] - Wp[:, :cin]).T
    bp = beta.astype(np.float64) - s * mean.astype(np.float64)
    return (np.ascontiguousarray(Wa, np.float32),
            np.ascontiguousarray(Wc, np.float32),
            bp.astype(np.float32))


def _prep_core(xp, W1a, W1c, b1, w2a_b, w2cb_b):
    xT = np.ascontiguousarray(xp.T)
    sq = np.einsum("nc,nc->n", xp.astype(np.float64), xp.astype(np.float64))
    bias_row = _tf32((-sq / 2).astype(np.float32))[None, :]
    xa = np.zeros((N, XA1), np.float32)
    xa[:, :C1] = xp
    xa[:, C1] = (-sq / 2).astype(np.float32)
    return dict(
        xTr=np.concatenate([_tf32(xT), bias_row], axis=0),
        xa1=xa,
        wa1=W1a, wc1=W1c,
        b1b=np.broadcast_to(b1, (P, O1)).copy(),
        w2a=w2a_b, w2cb=w2cb_b,
        onest=np.ones((1, P), np.float32),
        onesb=np.ones((1, P), ml_dtypes.bfloat16),
        io8=np.broadcast_to(np.arange(8, dtype=np.float32), (P, 8)).copy(),
    )


_LAST_EXEC_NS = {"l1": None}


def kernel(interm_repr, W1, bn1_gamma, bn1_beta, bn1_mean, bn1_var,
           W2, bn2_gamma, bn2_beta, bn2_mean, bn2_var, _trace=False):
    x = np.asarray(interm_repr, dtype=np.float32)
    (p,) = _get_programs()

    W1a, W1c, b1 = _fold_bn(np.asarray(W1), np.asarray(bn1_gamma),
                            np.asarray(bn1_beta), np.asarray(bn1_mean),
                            np.asarray(bn1_var), C1)
    W2a, W2c, b2 = _fold_bn(np.asarray(W2), np.asarray(bn2_gamma),
                            np.asarray(bn2_beta), np.asarray(bn2_mean),
                            np.asarray(bn2_var), C2)
    W1a = _tf32(W1a)
    W1c = _tf32(W1c)
    w2a_b = _bf16(W2a)
    w2cb_b = _bf16(np.concatenate([W2c, b2[None, :]], axis=0))

    in_maps = []
    for c in range(8):
        b, h = c // 2, c % 2
        perm = np.r_[h * HALF:(h + 1) * HALF, (1 - h) * HALF:(2 - h) * HALF]
        in_maps.append(_prep_core(x[b][perm], W1a, W1c, b1, w2a_b, w2cb_b))
    r = run_bass_kernel_spmd(p, in_maps, core_ids=list(range(8)), trace=_trace)
    _LAST_EXEC_NS["l1"] = r.exec_time_ns

    x2 = np.empty((B, N, O2), np.float32)
    for c in range(8):
        b, h = c // 2, c % 2
        x2[b, h * HALF:(h + 1) * HALF] = r.results[c]["xout"]
    return x2


if __name__ == "__main__":
    rng = np.random.default_rng(0)
    inp = dict(
        interm_repr=rng.standard_normal((B, N, C1), dtype=np.float32),
        W1=(rng.standard_normal((O1, 2 * C1)) / np.sqrt(2 * C1)).astype(np.float32),
        bn1_gamma=1 + 0.1 * rng.standard_normal(O1).astype(np.float32),
        bn1_beta=0.1 * rng.standard_normal(O1).astype(np.float32),
        bn1_mean=0.1 * rng.standard_normal(O1).astype(np.float32),
        bn1_var=0.5 + rng.random(O1).astype(np.float32),
        W2=(rng.standard_normal((O2, 2 * C2)) / np.sqrt(2 * C2)).astype(np.float32),
        bn2_gamma=1 + 0.1 * rng.standard_normal(O2).astype(np.float32),
        bn2_beta=0.1 * rng.standard_normal(O2).astype(np.float32),
        bn2_mean=0.1 * rng.standard_normal(O2).astype(np.float32),
        bn2_var=0.5 + rng.random(O2).astype(np.float32),
    )
    out = kernel(**inp)
    print("kernel out", out.shape, out.dtype, np.abs(out).mean())


# revision 31
# speedup vs baseline: 1.0126x; 1.0023x over previous
"""Trainium2 Bass kernel for PointCloudTeacher (2x EdgeConv with KNN graph).

Single fused launch on 8 NeuronCores. Core c handles (batch b = c//2,
row-half h = c%2); its inputs are row/column-permuted so its own 2048 rows
come first -- one SPMD program serves all cores. Block1 is replicated per
core pair (each core computes the full 4096-point cloud's block1) so block2
needs no cross-core exchange; block2 then handles only the core's own 2048
query rows. All block2 operands (x1^T in tf32, bf16 lhsT stripes, -|x|^2/2
bias row) are built on device, so there is no second launch and no host
round-trip.

Per block:
  - Coarse KNN keys via single-pass TF32 (float32r) gram matmul plus a K=1
    rank-1 pass adding -|x_m|^2/2; top-8 via DVE max8/find_index8.
  - Exact re-rank of the top-6 candidates: batched indirect gather of
    candidate rows (with a -sq/2 column) and dot against the center row
    [x_n, 1] -> exact f32 keys -> exact top-5 (slot 0 is always self).
  - y_a = (s*W_a) @ x for the full cloud; y_c = (s*W_c) @ x + b' computed
    at the epilogue (BN folded into the weights; max over k commutes with
    the monotone BN+LeakyReLU). Block1 y-matmuls are single-pass f32r
    (tf32); block2 y-matmuls are bf16 with a bf16 y_a table (output-only).
  - Epilogue: self row of y_a direct-loaded + 4 indirect gathers by the
    exact top-4 indices, max over k, add y_c, LeakyReLU via Prelu(0.2),
    and (block1) append the -|x1|^2/2 column for block2's rerank table.
"""

import numpy as np
import ml_dtypes

import concourse.bass as bass
import concourse.bacc as bacc
import concourse.mybir as mybir
from concourse import masks
from concourse.tile import TileContext
from concourse.bass_utils import run_bass_kernel_spmd

dt = mybir.dt
AF = mybir.ActivationFunctionType
OP = mybir.AluOpType

P = 128
N = 4096
HALF = 2048
B = 4
C1, O1 = 512, 864
C2, O2 = 864, 1728
K = 5
NCAND = 6            # coarse candidates kept for the exact re-rank
EPS = 1e-5
SLOPE = 0.2
XA1 = 516            # block1 rerank row: x row + -sq/2 + pad
XW = 868             # x1d row: x1 (864) + -sq/2 + pad
N_TILES = N // P     # 32
H_TILES = HALF // P  # 16

KS1 = [(0, 128), (128, 128), (256, 128), (384, 128)]            # C1 chunks
KS2 = [(i * 128, 128) for i in range(6)] + [(768, 96)]          # C2 chunks


def _tf32(a):
    a = np.ascontiguousarray(a, dtype=np.float32)
    u = a.view(np.uint32).astype(np.uint64)
    u = ((u + 0x1000 + ((u >> 13) & 1)) & 0xFFFFE000).astype(np.uint32)
    return u.view(np.float32)


def _bf16(a):
    return np.ascontiguousarray(a, dtype=np.float32).astype(ml_dtypes.bfloat16)


def _mid_bcast(ap, rep):
    """(P, F) access pattern -> (P, rep, F) with 0-stride middle dim."""
    pat = [list(ap.ap[0]), [0, rep], list(ap.ap[1])]
    return bass.AP(ap.tensor, ap.offset, pat)


def _last_bcast(ap, rep):
    """(P, F) access pattern -> (P, F, rep) with 0-stride last dim."""
    pat = [list(ap.ap[0]), list(ap.ap[1]), [0, rep]]
    return bass.AP(ap.tensor, ap.offset, pat)


# ---------------------------------------------------------------- program

def _build_fused():
    nc = bacc.Bacc("TRN2", target_bir_lowering=False)

    # ---- inputs (per-core; weights replicated)
    xTr = nc.dram_tensor("xTr", [C1 + 1, N], dt.float32r, kind="ExternalInput")
    xa1 = nc.dram_tensor("xa1", [N, XA1], dt.float32, kind="ExternalInput")
    wa1 = nc.dram_tensor("wa1", [C1, O1], dt.float32r, kind="ExternalInput")
    wc1 = nc.dram_tensor("wc1", [C1, O1], dt.float32r, kind="ExternalInput")
    b1b = nc.dram_tensor("b1b", [P, O1], dt.float32, kind="ExternalInput")
    w2a = nc.dram_tensor("w2a", [C2, O2], dt.bfloat16, kind="ExternalInput")
    w2cb = nc.dram_tensor("w2cb", [C2 + 1, O2], dt.bfloat16, kind="ExternalInput")
    onest = nc.dram_tensor("onest", [1, P], dt.float32r, kind="ExternalInput")
    onesb = nc.dram_tensor("onesb", [1, P], dt.bfloat16, kind="ExternalInput")
    io8 = nc.dram_tensor("io8", [P, 8], dt.float32, kind="ExternalInput")

    # ---- internal DRAM
    yad1 = nc.dram_tensor("yad1", [N, O1], dt.float32)
    x1d = nc.dram_tensor("x1d", [N, XW], dt.float32)
    x1bs = nc.dram_tensor("x1bs", [N_TILES, P, 7 * P], dt.bfloat16)
    yad2 = nc.dram_tensor("yad2", [N, O2], dt.bfloat16)
    xout = nc.dram_tensor("xout", [HALF, O2], dt.float32, kind="ExternalOutput")

    OS1 = [(0, 512), (512, 352)]
    OS2 = [(0, 512), (512, 352), (864, 512), (1376, 352)]

    with TileContext(nc) as tc:
        with tc.tile_pool(name="ppg", bufs=1) as ppg:
            onest_t = ppg.tile([1, P], dt.float32r, tag="onest")
            nc.sync.dma_start(onest_t[:], onest[:])
            onesb_t = ppg.tile([1, P], dt.bfloat16, tag="onesb")
            nc.sync.dma_start(onesb_t[:], onesb[:])
            io8t = ppg.tile([P, 8], dt.float32, tag="io8")
            nc.sync.dma_start(io8t[:], io8[:])

            pidx1_cm = tc.tile_pool(name="pidx1", bufs=N_TILES)
            pidx1 = pidx1_cm.__enter__()
            idx1_t = []
            pidx2_cm = tc.tile_pool(name="pidx2", bufs=H_TILES)
            pidx2 = pidx2_cm.__enter__()
            idx2_t = []

            # ---------------- shared rerank helper ----------------
            def rerank(pbs, pbc, t, keys, xa_dram, xa_w, cdata, idx_pool, idx_list):
                cs = slice(t * P, (t + 1) * P)
                top8 = pbs.tile([P, 8], dt.float32, tag="top8")
                nc.vector.max(out=top8[:], in_=keys[:])
                cidx = pbs.tile([P, 8], dt.uint32, tag="cidx")
                nc.vector.max_index(cidx[:], top8[:], keys[:])
                cidxf = pbs.tile([P, 8], dt.float32, tag="cidxf")
                nc.vector.tensor_copy(cidxf[:], cidx[:])

                cent = pbc.tile([P, xa_w], dt.float32, tag="cent")
                nc.sync.dma_start(cent[:], xa_dram[cs, :])
                ekeys = pbs.tile([P, 8], dt.float32, tag="ekeys")
                nc.vector.memset(ekeys[:, NCAND:8], -1e30)
                cand = pbc.tile([P, 3, xa_w], dt.float32, tag="cand")
                # slot 0 = self (copy of the center row, pre-memset)
                nc.scalar.activation(cand[:, 0, :], cent[:], AF.Copy)
                nc.vector.memset(cent[:, cdata:cdata + 1], 1.0)
                for (j0, jn) in ((0, 3), (3, 3)):
                    if j0 > 0:
                        cand = pbc.tile([P, 3, xa_w], dt.float32, tag="cand")
                    for j in range(jn):
                        if j0 + j == 0:
                            continue
                        nc.gpsimd.indirect_dma_start(
                            out=cand[:, j, :],
                            out_offset=None,
                            in_=xa_dram[:],
                            in_offset=bass.IndirectOffsetOnAxis(
                                ap=cidx[:, j0 + j:j0 + j + 1], axis=0
                            ),
                        )
                    nc.gpsimd.tensor_tensor(
                        out=cand[:, :jn, :], in0=cand[:, :jn, :],
                        in1=_mid_bcast(cent[:], jn), op=OP.mult,
                    )
                    nc.vector.tensor_reduce(
                        out=ekeys[:, j0:j0 + jn], in_=cand[:, :jn, :],
                        axis=mybir.AxisListType.X, op=OP.add,
                    )
                etop = pbs.tile([P, 8], dt.float32, tag="etop")
                nc.vector.max(out=etop[:], in_=ekeys[:])
                epos = pbs.tile([P, 8], dt.uint32, tag="epos")
                nc.vector.max_index(epos[:], etop[:], ekeys[:])
                eposf = pbs.tile([P, 8], dt.float32, tag="eposf")
                nc.vector.tensor_copy(eposf[:], epos[:])

                KR = K - 1
                m48 = pbs.tile([P, KR, NCAND], dt.float32, tag="m48")
                nc.vector.tensor_tensor(
                    out=m48[:], in0=_mid_bcast(io8t[:, 0:NCAND], KR),
                    in1=_last_bcast(eposf[:, 1:K], NCAND), op=OP.is_equal,
                )
                nc.vector.tensor_tensor(
                    out=m48[:], in0=m48[:],
                    in1=_mid_bcast(cidxf[:, 0:NCAND], KR), op=OP.mult,
                )
                idx4f = pbs.tile([P, KR], dt.float32, tag="idx4f")
                nc.vector.tensor_reduce(
                    out=idx4f[:], in_=m48[:], axis=mybir.AxisListType.X,
                    op=OP.add,
                )
                idx4 = idx_pool.tile([P, KR], dt.uint32, tag="idx4")
                nc.vector.tensor_copy(idx4[:], idx4f[:])
                idx_list.append(idx4)

            # ================ PHASE A: block1 gram/rerank/y_a ================
            with tc.tile_pool(name="pp1", bufs=1) as pp1:
                xtr_t = []
                for ci, (off, kk) in enumerate(KS1):
                    tt = pp1.tile([kk, N], dt.float32r, tag=f"xtr{ci}")
                    nc.sync.dma_start(tt[:], xTr[off:off + kk, :])
                    xtr_t.append(tt)
                brow1 = pp1.tile([1, N], dt.float32r, tag="brow1")
                nc.sync.dma_start(brow1[:], xTr[C1:C1 + 1, :])
                wa1_t, wc1_t = [], []
                for ci, (off, kk) in enumerate(KS1):
                    tt = pp1.tile([kk, O1], dt.float32r, tag=f"wa1{ci}")
                    nc.sync.dma_start(tt[:], wa1[off:off + kk, :])
                    wa1_t.append(tt)
                    tt = pp1.tile([kk, O1], dt.float32r, tag=f"wc1{ci}")
                    nc.sync.dma_start(tt[:], wc1[off:off + kk, :])
                    wc1_t.append(tt)
                b1t = pp1.tile([P, O1], dt.float32, tag="b1t")
                nc.sync.dma_start(b1t[:], b1b[:])

                with (
                    tc.tile_pool(name="pk", bufs=2) as pk,
                    tc.tile_pool(name="pbs", bufs=2) as pbs,
                    tc.tile_pool(name="pbc", bufs=2) as pbc,
                    tc.tile_pool(name="pac", bufs=2) as pac,
                    tc.tile_pool(name="pgq", bufs=2, space="PSUM") as pgq,
                    tc.tile_pool(name="py1", bufs=2, space="PSUM") as py1,
                ):
                    for t in range(N_TILES):
                        cs = slice(t * P, (t + 1) * P)
                        # ---- coarse gram keys in (128, 1024) psum quarters
                        keys = pk.tile([P, N], dt.float32, tag="keys")
                        for q in range(4):
                            ps = pgq.tile([P, 1024], dt.float32, tag="gps")
                            for si in range(2):
                                nsl = slice(q * 1024 + si * 512,
                                            q * 1024 + si * 512 + 512)
                                psl = slice(si * 512, si * 512 + 512)
                                for ci, (off, kk) in enumerate(KS1):
                                    nc.tensor.matmul(
                                        ps[:, psl],
                                        lhsT=xtr_t[ci][:kk, cs],
                                        rhs=xtr_t[ci][:kk, nsl],
                                        start=(ci == 0), stop=False,
                                        skip_group_check=True,
                                    )
                                nc.tensor.matmul(
                                    ps[:, psl],
                                    lhsT=onest_t[:],
                                    rhs=brow1[:, nsl],
                                    start=False, stop=True,
                                    skip_group_check=True,
                                )
                            nc.scalar.activation(
                                keys[:, q * 1024:(q + 1) * 1024], ps[:], AF.Copy
                            )
                        # ---- top-6 + exact rerank -> idx4
                        rerank(pbs, pbc, t, keys, xa1, XA1, C1, pidx1, idx1_t)
                        # ---- y1_a tile (interleaved)
                        ps = py1.tile([P, O1], dt.float32, tag="yps")
                        for (oo, ow) in OS1:
                            for ci, (off, kk) in enumerate(KS1):
                                nc.tensor.matmul(
                                    ps[:, oo:oo + ow],
                                    lhsT=xtr_t[ci][:kk, cs],
                                    rhs=wa1_t[ci][:, oo:oo + ow],
                                    start=(ci == 0), stop=(ci == len(KS1) - 1),
                                    skip_group_check=True,
                                )
                        sb = pac.tile([P, O1], dt.float32, tag="ya_sb")
                        nc.scalar.activation(sb[:], ps[:], AF.Copy)
                        nc.sync.dma_start(yad1[cs, :], sb[:])

                tc.strict_bb_all_engine_barrier()

                # ================ PHASE B: block1 epilogue ================
                with (
                    tc.tile_pool(name="pg5", bufs=2) as pg5,
                    tc.tile_pool(name="pe2", bufs=2) as pe2,
                    tc.tile_pool(name="pyc", bufs=2, space="PSUM") as pyc,
                ):
                    for t in range(N_TILES):
                        cs = slice(t * P, (t + 1) * P)
                        # y1_c for this tile (psum -> +b1 -> sbuf)
                        ps = pyc.tile([P, O1], dt.float32, tag="ycps")
                        for (oo, ow) in OS1:
                            for ci, (off, kk) in enumerate(KS1):
                                nc.tensor.matmul(
                                    ps[:, oo:oo + ow],
                                    lhsT=xtr_t[ci][:kk, cs],
                                    rhs=wc1_t[ci][:, oo:oo + ow],
                                    start=(ci == 0), stop=(ci == len(KS1) - 1),
                                    skip_group_check=True,
                                )
                        yct = pe2.tile([P, O1], dt.float32, tag="yct")
                        nc.vector.tensor_tensor(out=yct[:], in0=ps[:],
                                                in1=b1t[:], op=OP.add)
                        # gather the 5 neighbor rows of y_a
                        g5 = pg5.tile([P, K, O1], dt.float32, tag="g5")
                        nc.sync.dma_start(g5[:, 0, :], yad1[cs, :])
                        for j in range(K - 1):
                            nc.gpsimd.indirect_dma_start(
                                out=g5[:, j + 1, :],
                                out_offset=None,
                                in_=yad1[:],
                                in_offset=bass.IndirectOffsetOnAxis(
                                    ap=idx1_t[t][:, j:j + 1], axis=0
                                ),
                            )
                        mb = pe2.tile([P, O1], dt.float32, tag="mb")
                        nc.vector.tensor_tensor(out=mb[:], in0=g5[:, 0, :],
                                                in1=g5[:, 1, :], op=OP.max)
                        for j in range(2, K):
                            nc.vector.tensor_tensor(out=mb[:], in0=mb[:],
                                                    in1=g5[:, j, :], op=OP.max)
                        xx = pe2.tile([P, XW], dt.float32, tag="xx")
                        nc.vector.tensor_tensor(out=xx[:, 0:O1], in0=mb[:],
                                                in1=yct[:], op=OP.add)
                        nc.scalar.activation(xx[:, 0:O1], xx[:, 0:O1],
                                             AF.Prelu, alpha=SLOPE)
                        # -|x1|^2/2 column + zero pad
                        sq = pe2.tile([P, O1], dt.float32, tag="sqscratch")
                        nc.scalar.activation(sq[:], xx[:, 0:O1], AF.Square,
                                             accum_out=xx[:, O1:O1 + 1])
                        nc.vector.tensor_scalar_mul(xx[:, O1:O1 + 1],
                                                    xx[:, O1:O1 + 1], -0.5)
                        nc.vector.memset(xx[:, O1 + 1:XW], 0.0)
                        nc.sync.dma_start(x1d[cs, :], xx[:])

            # ================ PHASE P2: transpose x1 -> x1T + stripes =======
            with tc.tile_pool(name="pw2", bufs=1) as pw2:
                w2a_t = []
                for ci, (off, kk) in enumerate(KS2):
                    tt = pw2.tile([kk, O2], dt.bfloat16, tag=f"w2a{ci}")
                    nc.sync.dma_start(tt[:], w2a[off:off + kk, :])
                    w2a_t.append(tt)
                tc.strict_bb_all_engine_barrier()

                px1T_cm = tc.tile_pool(name="px1T", bufs=1)
                px1T = px1T_cm.__enter__()
                x1T = []
                for j in range(7):
                    rows = 128 if j < 6 else 96
                    x1T.append(px1T.tile([rows, N], dt.float32r,
                                         tag=f"x1T{j}", name=f"x1T{j}"))
                # -|x1|^2/2 keys-bias row, base partition 0 (PE operand)
                brow2 = px1T.tile([1, N], dt.float32r, tag="brow2", name="brow2")

                with (
                    tc.tile_pool(name="pld", bufs=2) as pld,
                    tc.tile_pool(name="pstb", bufs=2) as pstb,
                    tc.tile_pool(name="ptp", bufs=4, space="PSUM") as ptp,
                ):
                    ident = pld.tile([P, P], dt.float32, tag="ident")
                    masks.make_identity(nc, ident[:])
                    for t in range(N_TILES):
                        cs = slice(t * P, (t + 1) * P)
                        xx = pld.tile([P, XW], dt.float32, tag="xxl")
                        nc.sync.dma_start(xx[:], x1d[cs, :])
                        stb = pstb.tile([P, 7 * P], dt.bfloat16, tag="stb")
                        nc.gpsimd.memset(stb[96:128, 6 * P:7 * P], 0.0)
                        for j in range(7):
                            w = 128 if j < 6 else 97
                            kd = 128 if j < 6 else 96
                            ps = ptp.tile([P, P], dt.float32, tag="tps")
                            nc.tensor.matmul(
                                ps[:w, :], lhsT=xx[:, j * P:j * P + w],
                                rhs=ident[:], is_transpose=True,
                                start=True, stop=True, skip_group_check=True,
                            )
                            nc.scalar.activation(
                                x1T[j][:kd, cs], ps[:kd, :], AF.Copy
                            )
                            if j == 6:
                                nc.scalar.activation(
                                    brow2[:, cs], ps[96:97, :], AF.Copy
                                )
                            nc.vector.tensor_copy(
                                stb[:kd, j * P:(j + 1) * P], ps[:kd, :]
                            )
                        nc.sync.dma_start(x1bs[t], stb[:])

                tc.strict_bb_all_engine_barrier()

                # ================ PHASE C: block2 gram/rerank/y_a ============
                with (
                    tc.tile_pool(name="pk2", bufs=2) as pk2,
                    tc.tile_pool(name="pbs2", bufs=2) as pbs2,
                    tc.tile_pool(name="pbc2", bufs=1) as pbc2,
                    tc.tile_pool(name="pstr", bufs=2) as pstr,
                    tc.tile_pool(name="pac2", bufs=1) as pac2,
                    tc.tile_pool(name="pgq2", bufs=2, space="PSUM") as pgq2,
                    tc.tile_pool(name="py2", bufs=2, space="PSUM") as py2,
                ):
                    def y2a_tile(yt):
                        cs = slice(yt * P, (yt + 1) * P)
                        stq = pstr.tile([P, 7 * P], dt.bfloat16, tag="stq")
                        nc.sync.dma_start(stq[:], x1bs[yt])
                        sb = pac2.tile([P, O2], dt.bfloat16, tag="y2a_sb")
                        for hh in range(2):
                            ps = py2.tile([P, 864], dt.float32, tag="y2ps",
                                          name=f"y2ps{hh}")
                            for (oo, ow) in OS2[hh * 2:hh * 2 + 2]:
                                po = oo - hh * 864
                                for ci, (off, kk) in enumerate(KS2):
                                    nc.tensor.matmul(
                                        ps[:, po:po + ow],
                                        lhsT=stq[:kk, ci * P:(ci + 1) * P],
                                        rhs=w2a_t[ci][:, oo:oo + ow],
                                        start=(ci == 0), stop=(ci == len(KS2) - 1),
                                        skip_group_check=True,
                                    )
                            nc.scalar.activation(
                                sb[:, hh * 864:(hh + 1) * 864], ps[:], AF.Copy
                            )
                        nc.sync.dma_start(yad2[cs, :], sb[:])

                    for t in range(H_TILES):
                        cs = slice(t * P, (t + 1) * P)
                        keys = pk2.tile([P, N], dt.float32, tag="keys2")
                        for q in range(4):
                            ps = pgq2.tile([P, 1024], dt.float32, tag="gps2")
                            for si in range(2):
                                nsl = slice(q * 1024 + si * 512,
                                            q * 1024 + si * 512 + 512)
                                psl = slice(si * 512, si * 512 + 512)
                                for ci, (off, kk) in enumerate(KS2):
                                    kd = 128 if ci < 6 else 96
                                    nc.tensor.matmul(
                                        ps[:, psl],
                                        lhsT=x1T[ci][:kd, cs],
                                        rhs=x1T[ci][:kd, nsl],
                                        start=(ci == 0), stop=False,
                                        skip_group_check=True,
                                    )
                                nc.tensor.matmul(
                                    ps[:, psl],
                                    lhsT=onest_t[:],
                                    rhs=brow2[:, nsl],
                                    start=False, stop=True,
                                    skip_group_check=True,
                                )
                            nc.scalar.activation(
                                keys[:, q * 1024:(q + 1) * 1024], ps[:], AF.Copy
                            )
                        rerank(pbs2, pbc2, t, keys, x1d, XW, C2, pidx2, idx2_t)
                        y2a_tile(2 * t)
                        y2a_tile(2 * t + 1)

                px1T_cm.__exit__(None, None, None)

                # ================ PHASE D: block2 epilogue ===================
                with (
                    tc.tile_pool(name="pwc2", bufs=1) as pwc2,
                    tc.tile_pool(name="pstr2", bufs=2) as pstr2,
                    tc.tile_pool(name="pg5b", bufs=2) as pg5b,
                    tc.tile_pool(name="pe2b", bufs=2) as pe2b,
                    tc.tile_pool(name="pyc2", bufs=2, space="PSUM") as pyc2,
                ):
                    w2c_t = []
                    for ci, (off, kk) in enumerate(KS2):
                        tt = pwc2.tile([kk, O2], dt.bfloat16, tag=f"w2c{ci}")
                        nc.sync.dma_start(tt[:], w2cb[off:off + kk, :])
                        w2c_t.append(tt)
                    b2row = pwc2.tile([1, O2], dt.bfloat16, tag="b2row")
                    nc.sync.dma_start(b2row[:], w2cb[C2:C2 + 1, :])
                    tc.strict_bb_all_engine_barrier()
                    for t in range(H_TILES):
                        cs = slice(t * P, (t + 1) * P)
                        stq = pstr2.tile([P, 7 * P], dt.bfloat16, tag="stq2")
                        nc.sync.dma_start(stq[:], x1bs[t])
                        yct = pe2b.tile([P, O2], dt.float32, tag="yct2")
                        for hh in range(2):
                            ps = pyc2.tile([P, 864], dt.float32, tag="yc2ps",
                                           name=f"yc2ps{hh}")
                            for (oo, ow) in OS2[hh * 2:hh * 2 + 2]:
                                po = oo - hh * 864
                                for ci, (off, kk) in enumerate(KS2):
                                    nc.tensor.matmul(
                                        ps[:, po:po + ow],
                                        lhsT=stq[:kk, ci * P:(ci + 1) * P],
                                        rhs=w2c_t[ci][:, oo:oo + ow],
                                        start=(ci == 0), stop=False,
                                        skip_group_check=True,
                                    )
                                nc.tensor.matmul(
                                    ps[:, po:po + ow],
                                    lhsT=onesb_t[:],
                                    rhs=b2row[:, oo:oo + ow],
                                    start=False, stop=True,
                                    skip_group_check=True,
                                )
                            nc.scalar.activation(
                                yct[:, hh * 864:(hh + 1) * 864], ps[:], AF.Copy
                            )
                        g5 = pg5b.tile([P, K, O2], dt.bfloat16, tag="g5b")
                        nc.sync.dma_start(g5[:, 0, :], yad2[cs, :])
                        for j in range(K - 1):
                            nc.gpsimd.indirect_dma_start(
                                out=g5[:, j + 1, :],
                                out_offset=None,
                                in_=yad2[:],
                                in_offset=bass.IndirectOffsetOnAxis(
                                    ap=idx2_t[t][:, j:j + 1], axis=0
                                ),
                            )
                        mb = pe2b.tile([P, O2], dt.bfloat16, tag="mb2")
                        nc.vector.tensor_tensor(out=mb[:], in0=g5[:, 0, :],
                                                in1=g5[:, 1, :], op=OP.max)
                        for j in range(2, K):
                            nc.vector.tensor_tensor(out=mb[:], in0=mb[:],
                                                    in1=g5[:, j, :], op=OP.max)
                        xo = pe2b.tile([P, O2], dt.float32, tag="xo")
                        nc.vector.tensor_tensor(out=xo[:], in0=mb[:],
                                                in1=yct[:], op=OP.add)
                        nc.scalar.activation(xo[:], xo[:], AF.Prelu, alpha=SLOPE)
                        nc.sync.dma_start(xout[cs, :], xo[:])

            pidx2_cm.__exit__(None, None, None)
            pidx1_cm.__exit__(None, None, None)

    nc.finalize()
    return nc


_CACHE = {}


def _get_programs():
    if "p" not in _CACHE:
        _CACHE["p"] = _build_fused()
    return (_CACHE["p"],)


# ---------------------------------------------------------------- host side

def _fold_bn(W, gamma, beta, mean, var, cin):
    s = gamma.astype(np.float64) / np.sqrt(var.astype(np.float64) + EPS)
    Wp = s[:, None] * W.astype(np.float64)
    Wa = Wp[:, :cin].T
    Wc = (Wp[:, cin:] - Wp[:, :cin]).T
    bp = beta.astype(np.float64) - s * mean.astype(np.float64)
    return (np.ascontiguousarray(Wa, np.float32),
            np.ascontiguousarray(Wc, np.float32),
            bp.astype(np.float32))


def _prep_core(xp, W1a, W1c, b1, w2a_b, w2cb_b):
    xT = np.ascontiguousarray(xp.T)
    sq = np.einsum("nc,nc->n", xp.astype(np.float64), xp.astype(np.float64))
    bias_row = _tf32((-sq / 2).astype(np.float32))[None, :]
    xa = np.zeros((N, XA1), np.float32)
    xa[:, :C1] = xp
    xa[:, C1] = (-sq / 2).astype(np.float32)
    return dict(
        xTr=np.concatenate([_tf32(xT), bias_row], axis=0),
        xa1=xa,
        wa1=W1a, wc1=W1c,
        b1b=np.broadcast_to(b1, (P, O1)).copy(),
        w2a=w2a_b, w2cb=w2cb_b,
        onest=np.ones((1, P), np.float32),
        onesb=np.ones((1, P), ml_dtypes.bfloat16),
        io8=np.broadcast_to(np.arange(8, dtype=np.float32), (P, 8)).copy(),
    )


_LAST_EXEC_NS = {"l1": None}


def kernel(interm_repr, W1, bn1_gamma, bn1_beta, bn1_mean, bn1_var,
           W2, bn2_gamma, bn2_beta, bn2_mean, bn2_var, _trace=False):
    x = np.asarray(interm_repr, dtype=np.float32)
    (p,) = _get_programs()

    W1a, W1c, b1 = _fold_bn(np.asarray(W1), np.asarray(bn1_gamma),
                            np.asarray(bn1_beta), np.asarray(bn1_mean),
                            np.asarray(bn1_var), C1)
    W2a, W2c, b2 = _fold_bn(np.asarray(W2), np.asarray(bn2_gamma),
                            np.asarray(bn2_beta), np.asarray(bn2_mean),
                            np.asarray(bn2_var), C2)
    W1a = _tf32(W1a)
    W1c = _tf32(W1c)
    w2a_b = _bf16(W2a)
    w2cb_b = _bf16(np.concatenate([W2c, b2[None, :]], axis=0))

    in_maps = []
    for c in range(8):
        b, h = c // 2, c % 2
        perm = np.r_[h * HALF:(h + 1) * HALF, (1 - h) * HALF:(2 - h) * HALF]
        in_maps.append(_prep_core(x[b][perm], W1a, W1c, b1, w2a_b, w2cb_b))
    r = run_bass_kernel_spmd(p, in_maps, core_ids=list(range(8)), trace=_trace)
    _LAST_EXEC_NS["l1"] = r.exec_time_ns

    x2 = np.empty((B, N, O2), np.float32)
    for c in range(8):
        b, h = c // 2, c % 2
        x2[b, h * HALF:(h + 1) * HALF] = r.results[c]["xout"]
    return x2


if __name__ == "__main__":
    rng = np.random.default_rng(0)
    inp = dict(
        interm_repr=rng.standard_normal((B, N, C1), dtype=np.float32),
        W1=(rng.standard_normal((O1, 2 * C1)) / np.sqrt(2 * C1)).astype(np.float32),
        bn1_gamma=1 + 0.1 * rng.standard_normal(O1).astype(np.float32),
        bn1_beta=0.1 * rng.standard_normal(O1).astype(np.float32),
        bn1_mean=0.1 * rng.standard_normal(O1).astype(np.float32),
        bn1_var=0.5 + rng.random(O1).astype(np.float32),
        W2=(rng.standard_normal((O2, 2 * C2)) / np.sqrt(2 * C2)).astype(np.float32),
        bn2_gamma=1 + 0.1 * rng.standard_normal(O2).astype(np.float32),
        bn2_beta=0.1 * rng.standard_normal(O2).astype(np.float32),
        bn2_mean=0.1 * rng.standard_normal(O2).astype(np.float32),
        bn2_var=0.5 + rng.random(O2).astype(np.float32),
    )
    out = kernel(**inp)
    print("kernel out", out.shape, out.dtype, np.abs(out).mean())


# revision 34
# speedup vs baseline: 1.0155x; 1.0028x over previous
"""Trainium2 Bass kernel for PointCloudTeacher (2x EdgeConv with KNN graph).

Single fused launch on 8 NeuronCores. Core c handles (batch b = c//2,
row-half h = c%2); its inputs are row/column-permuted so its own 2048 rows
come first -- one SPMD program serves all cores. Block1 is replicated per
core pair (each core computes the full 4096-point cloud's block1) so block2
needs no cross-core exchange; block2 then handles only the core's own 2048
query rows. All block2 operands (x1^T in tf32, bf16 lhsT stripes, -|x|^2/2
bias row) are built on device, so there is no second launch and no host
round-trip.

Per block:
  - Coarse KNN keys via single-pass TF32 (float32r) gram matmul plus a K=1
    rank-1 pass adding -|x_m|^2/2; top-8 via DVE max8/find_index8.
  - Exact re-rank of the top-6 candidates: batched indirect gather of
    candidate rows (with a -sq/2 column) and dot against the center row
    [x_n, 1] -> exact f32 keys -> exact top-5 (slot 0 is always self).
  - y_a = (s*W_a) @ x for the full cloud; y_c = (s*W_c) @ x + b' computed
    at the epilogue (BN folded into the weights; max over k commutes with
    the monotone BN+LeakyReLU). Block1 y-matmuls are single-pass f32r
    (tf32); block2 y-matmuls are bf16 with a bf16 y_a table (output-only).
  - Epilogue: self row of y_a direct-loaded + 4 indirect gathers by the
    exact top-4 indices, max over k, add y_c, LeakyReLU via Prelu(0.2),
    and (block1) append the -|x1|^2/2 column for block2's rerank table.
"""

import numpy as np
import ml_dtypes

import concourse.bass as bass
import concourse.bacc as bacc
import concourse.mybir as mybir
from concourse import masks
from concourse.tile import TileContext
from concourse.bass_utils import run_bass_kernel_spmd

dt = mybir.dt
AF = mybir.ActivationFunctionType
OP = mybir.AluOpType

P = 128
N = 4096
HALF = 2048
B = 4
C1, O1 = 512, 864
C2, O2 = 864, 1728
K = 5
NCAND = 6            # coarse candidates kept for the exact re-rank
EPS = 1e-5
SLOPE = 0.2
XA1 = 516            # block1 rerank row: x row + -sq/2 + pad
XW = 868             # x1d row: x1 (864) + -sq/2 + pad
N_TILES = N // P     # 32
H_TILES = HALF // P  # 16

KS1 = [(0, 128), (128, 128), (256, 128), (384, 128)]            # C1 chunks
KS2 = [(i * 128, 128) for i in range(6)] + [(768, 96)]          # C2 chunks


def _tf32(a):
    a = np.ascontiguousarray(a, dtype=np.float32)
    u = a.view(np.uint32).astype(np.uint64)
    u = ((u + 0x1000 + ((u >> 13) & 1)) & 0xFFFFE000).astype(np.uint32)
    return u.view(np.float32)


def _bf16(a):
    return np.ascontiguousarray(a, dtype=np.float32).astype(ml_dtypes.bfloat16)


def _mid_bcast(ap, rep):
    """(P, F) access pattern -> (P, rep, F) with 0-stride middle dim."""
    pat = [list(ap.ap[0]), [0, rep], list(ap.ap[1])]
    return bass.AP(ap.tensor, ap.offset, pat)


def _last_bcast(ap, rep):
    """(P, F) access pattern -> (P, F, rep) with 0-stride last dim."""
    pat = [list(ap.ap[0]), list(ap.ap[1]), [0, rep]]
    return bass.AP(ap.tensor, ap.offset, pat)


# ---------------------------------------------------------------- program

def _build_fused():
    nc = bacc.Bacc("TRN2", target_bir_lowering=False)

    # ---- inputs (per-core; weights replicated)
    xTr = nc.dram_tensor("xTr", [C1 + 1, N], dt.float32r, kind="ExternalInput")
    xa1 = nc.dram_tensor("xa1", [N, XA1], dt.float32, kind="ExternalInput")
    wa1 = nc.dram_tensor("wa1", [C1, O1], dt.float32r, kind="ExternalInput")
    wc1 = nc.dram_tensor("wc1", [C1, O1], dt.float32r, kind="ExternalInput")
    b1b = nc.dram_tensor("b1b", [P, O1], dt.float32, kind="ExternalInput")
    w2a = nc.dram_tensor("w2a", [C2, O2], dt.bfloat16, kind="ExternalInput")
    w2cb = nc.dram_tensor("w2cb", [C2 + 1, O2], dt.bfloat16, kind="ExternalInput")
    onest = nc.dram_tensor("onest", [1, P], dt.float32r, kind="ExternalInput")
    onesb = nc.dram_tensor("onesb", [1, P], dt.bfloat16, kind="ExternalInput")
    io8 = nc.dram_tensor("io8", [P, 8], dt.float32, kind="ExternalInput")

    # ---- internal DRAM
    yad1 = nc.dram_tensor("yad1", [N, O1], dt.float32)
    x1d = nc.dram_tensor("x1d", [N, XW], dt.float32)
    x1bs = nc.dram_tensor("x1bs", [N_TILES, P, 7 * P], dt.bfloat16)
    yad2 = nc.dram_tensor("yad2", [N, O2], dt.bfloat16)
    xout = nc.dram_tensor("xout", [HALF, O2], dt.float32, kind="ExternalOutput")

    OS1 = [(0, 512), (512, 352)]
    OS2 = [(0, 512), (512, 352), (864, 512), (1376, 352)]

    with TileContext(nc) as tc:
        with tc.tile_pool(name="ppg", bufs=1) as ppg:
            onest_t = ppg.tile([1, P], dt.float32r, tag="onest")
            nc.sync.dma_start(onest_t[:], onest[:])
            onesb_t = ppg.tile([1, P], dt.bfloat16, tag="onesb")
            nc.sync.dma_start(onesb_t[:], onesb[:])
            io8t = ppg.tile([P, 8], dt.float32, tag="io8")
            nc.sync.dma_start(io8t[:], io8[:])

            pidx1_cm = tc.tile_pool(name="pidx1", bufs=N_TILES)
            pidx1 = pidx1_cm.__enter__()
            idx1_t = []
            pidx2_cm = tc.tile_pool(name="pidx2", bufs=H_TILES)
            pidx2 = pidx2_cm.__enter__()
            idx2_t = []

            # ---------------- shared rerank helper ----------------
            def rerank(pbs, pbc, t, keys, xa_dram, xa_w, cdata, idx_pool, idx_list):
                cs = slice(t * P, (t + 1) * P)
                top8 = pbs.tile([P, 8], dt.float32, tag="top8")
                nc.vector.max(out=top8[:], in_=keys[:])
                cidx = pbs.tile([P, 8], dt.uint32, tag="cidx")
                nc.vector.max_index(cidx[:], top8[:], keys[:])
                cidxf = pbs.tile([P, 8], dt.float32, tag="cidxf")
                nc.vector.tensor_copy(cidxf[:], cidx[:])

                cent = pbc.tile([P, xa_w], dt.float32, tag="cent")
                nc.sync.dma_start(cent[:], xa_dram[cs, :])
                ekeys = pbs.tile([P, 8], dt.float32, tag="ekeys")
                nc.vector.memset(ekeys[:, NCAND:8], -1e30)
                cand = pbc.tile([P, 3, xa_w], dt.float32, tag="cand")
                # slot 0 = self (copy of the center row, pre-memset)
                nc.scalar.activation(cand[:, 0, :], cent[:], AF.Copy)
                nc.vector.memset(cent[:, cdata:cdata + 1], 1.0)
                for (j0, jn) in ((0, 3), (3, 3)):
                    if j0 > 0:
                        cand = pbc.tile([P, 3, xa_w], dt.float32, tag="cand")
                    for j in range(jn):
                        if j0 + j == 0:
                            continue
                        nc.gpsimd.indirect_dma_start(
                            out=cand[:, j, :],
                            out_offset=None,
                            in_=xa_dram[:],
                            in_offset=bass.IndirectOffsetOnAxis(
                                ap=cidx[:, j0 + j:j0 + j + 1], axis=0
                            ),
                        )
                    nc.gpsimd.tensor_tensor(
                        out=cand[:, :jn, :], in0=cand[:, :jn, :],
                        in1=_mid_bcast(cent[:], jn), op=OP.mult,
                    )
                    nc.vector.tensor_reduce(
                        out=ekeys[:, j0:j0 + jn], in_=cand[:, :jn, :],
                        axis=mybir.AxisListType.X, op=OP.add,
                    )
                etop = pbs.tile([P, 8], dt.float32, tag="etop")
                nc.vector.max(out=etop[:], in_=ekeys[:])
                epos = pbs.tile([P, 8], dt.uint32, tag="epos")
                nc.vector.max_index(epos[:], etop[:], ekeys[:])
                eposf = pbs.tile([P, 8], dt.float32, tag="eposf")
                nc.vector.tensor_copy(eposf[:], epos[:])

                KR = K - 1
                m48 = pbs.tile([P, KR, NCAND], dt.float32, tag="m48")
                nc.vector.tensor_tensor(
                    out=m48[:], in0=_mid_bcast(io8t[:, 0:NCAND], KR),
                    in1=_last_bcast(eposf[:, 1:K], NCAND), op=OP.is_equal,
                )
                nc.vector.tensor_tensor(
                    out=m48[:], in0=m48[:],
                    in1=_mid_bcast(cidxf[:, 0:NCAND], KR), op=OP.mult,
                )
                idx4f = pbs.tile([P, KR], dt.float32, tag="idx4f")
                nc.vector.tensor_reduce(
                    out=idx4f[:], in_=m48[:], axis=mybir.AxisListType.X,
                    op=OP.add,
                )
                idx4 = idx_pool.tile([P, KR], dt.uint32, tag="idx4")
                nc.vector.tensor_copy(idx4[:], idx4f[:])
                idx_list.append(idx4)

            # ================ PHASE A: block1 gram/rerank/y_a ================
            with tc.tile_pool(name="pp1", bufs=1) as pp1:
                xtr_t = []
                for ci, (off, kk) in enumerate(KS1):
                    tt = pp1.tile([kk, N], dt.float32r, tag=f"xtr{ci}")
                    nc.sync.dma_start(tt[:], xTr[off:off + kk, :])
                    xtr_t.append(tt)
                brow1 = pp1.tile([1, N], dt.float32r, tag="brow1")
                nc.sync.dma_start(brow1[:], xTr[C1:C1 + 1, :])
                wa1_t, wc1_t = [], []
                for ci, (off, kk) in enumerate(KS1):
                    tt = pp1.tile([kk, O1], dt.float32r, tag=f"wa1{ci}")
                    nc.sync.dma_start(tt[:], wa1[off:off + kk, :])
                    wa1_t.append(tt)
                    tt = pp1.tile([kk, O1], dt.float32r, tag=f"wc1{ci}")
                    nc.sync.dma_start(tt[:], wc1[off:off + kk, :])
                    wc1_t.append(tt)
                b1t = pp1.tile([P, O1], dt.float32, tag="b1t")
                nc.sync.dma_start(b1t[:], b1b[:])

                with (
                    tc.tile_pool(name="pk", bufs=2) as pk,
                    tc.tile_pool(name="pbs", bufs=2) as pbs,
                    tc.tile_pool(name="pbc", bufs=2) as pbc,
                    tc.tile_pool(name="pac", bufs=2) as pac,
                    tc.tile_pool(name="pgq", bufs=2, space="PSUM") as pgq,
                    tc.tile_pool(name="py1", bufs=2, space="PSUM") as py1,
                ):
                    for t in range(N_TILES):
                        cs = slice(t * P, (t + 1) * P)
                        # ---- coarse gram keys in (128, 1024) psum quarters
                        keys = pk.tile([P, N], dt.float32, tag="keys")
                        for q in range(4):
                            ps = pgq.tile([P, 1024], dt.float32, tag="gps")
                            for si in range(2):
                                nsl = slice(q * 1024 + si * 512,
                                            q * 1024 + si * 512 + 512)
                                psl = slice(si * 512, si * 512 + 512)
                                for ci, (off, kk) in enumerate(KS1):
                                    nc.tensor.matmul(
                                        ps[:, psl],
                                        lhsT=xtr_t[ci][:kk, cs],
                                        rhs=xtr_t[ci][:kk, nsl],
                                        start=(ci == 0), stop=False,
                                        skip_group_check=True,
                                    )
                                nc.tensor.matmul(
                                    ps[:, psl],
                                    lhsT=onest_t[:],
                                    rhs=brow1[:, nsl],
                                    start=False, stop=True,
                                    skip_group_check=True,
                                )
                            nc.scalar.activation(
                                keys[:, q * 1024:(q + 1) * 1024], ps[:], AF.Copy
                            )
                        # ---- top-6 + exact rerank -> idx4
                        rerank(pbs, pbc, t, keys, xa1, XA1, C1, pidx1, idx1_t)
                        # ---- y1_a tile (interleaved)
                        ps = py1.tile([P, O1], dt.float32, tag="yps")
                        for (oo, ow) in OS1:
                            for ci, (off, kk) in enumerate(KS1):
                                nc.tensor.matmul(
                                    ps[:, oo:oo + ow],
                                    lhsT=xtr_t[ci][:kk, cs],
                                    rhs=wa1_t[ci][:, oo:oo + ow],
                                    start=(ci == 0), stop=(ci == len(KS1) - 1),
                                    skip_group_check=True,
                                )
                        sb = pac.tile([P, O1], dt.float32, tag="ya_sb")
                        nc.scalar.activation(sb[:], ps[:], AF.Copy)
                        nc.sync.dma_start(yad1[cs, :], sb[:])

                tc.strict_bb_all_engine_barrier()

                # ================ PHASE B: block1 epilogue ================
                with (
                    tc.tile_pool(name="pg5", bufs=2) as pg5,
                    tc.tile_pool(name="pe2", bufs=2) as pe2,
                    tc.tile_pool(name="pyc", bufs=2, space="PSUM") as pyc,
                ):
                    for t in range(N_TILES):
                        cs = slice(t * P, (t + 1) * P)
                        # y1_c for this tile (psum -> +b1 -> sbuf)
                        ps = pyc.tile([P, O1], dt.float32, tag="ycps")
                        for (oo, ow) in OS1:
                            for ci, (off, kk) in enumerate(KS1):
                                nc.tensor.matmul(
                                    ps[:, oo:oo + ow],
                                    lhsT=xtr_t[ci][:kk, cs],
                                    rhs=wc1_t[ci][:, oo:oo + ow],
                                    start=(ci == 0), stop=(ci == len(KS1) - 1),
                                    skip_group_check=True,
                                )
                        yct = pe2.tile([P, O1], dt.float32, tag="yct")
                        nc.vector.tensor_tensor(out=yct[:], in0=ps[:],
                                                in1=b1t[:], op=OP.add)
                        # gather the 5 neighbor rows of y_a
                        g5 = pg5.tile([P, K, O1], dt.float32, tag="g5")
                        nc.sync.dma_start(g5[:, 0, :], yad1[cs, :])
                        for j in range(K - 1):
                            nc.gpsimd.indirect_dma_start(
                                out=g5[:, j + 1, :],
                                out_offset=None,
                                in_=yad1[:],
                                in_offset=bass.IndirectOffsetOnAxis(
                                    ap=idx1_t[t][:, j:j + 1], axis=0
                                ),
                            )
                        mb = pe2.tile([P, O1], dt.float32, tag="mb")
                        nc.vector.tensor_tensor(out=mb[:], in0=g5[:, 0, :],
                                                in1=g5[:, 1, :], op=OP.max)
                        for j in range(2, K):
                            nc.vector.tensor_tensor(out=mb[:], in0=mb[:],
                                                    in1=g5[:, j, :], op=OP.max)
                        xx = pe2.tile([P, XW], dt.float32, tag="xx")
                        nc.vector.tensor_tensor(out=xx[:, 0:O1], in0=mb[:],
                                                in1=yct[:], op=OP.add)
                        nc.scalar.activation(xx[:, 0:O1], xx[:, 0:O1],
                                             AF.Prelu, alpha=SLOPE)
                        # -|x1|^2/2 column + zero pad
                        sq = pe2.tile([P, O1], dt.float32, tag="sqscratch")
                        nc.scalar.activation(sq[:], xx[:, 0:O1], AF.Square,
                                             accum_out=xx[:, O1:O1 + 1])
                        nc.vector.tensor_scalar_mul(xx[:, O1:O1 + 1],
                                                    xx[:, O1:O1 + 1], -0.5)
                        nc.vector.memset(xx[:, O1 + 1:XW], 0.0)
                        nc.sync.dma_start(x1d[cs, :], xx[:])

            # ================ PHASE P2: transpose x1 -> x1T + stripes =======
            with tc.tile_pool(name="pw2", bufs=1) as pw2:
                w2a_t = []
                for ci, (off, kk) in enumerate(KS2):
                    tt = pw2.tile([kk, O2], dt.bfloat16, tag=f"w2a{ci}")
                    nc.sync.dma_start(tt[:], w2a[off:off + kk, :])
                    w2a_t.append(tt)
                tc.strict_bb_all_engine_barrier()

                px1T_cm = tc.tile_pool(name="px1T", bufs=1)
                px1T = px1T_cm.__enter__()
                x1T = []
                for j in range(7):
                    rows = 128 if j < 6 else 96
                    x1T.append(px1T.tile([rows, N], dt.float32r,
                                         tag=f"x1T{j}", name=f"x1T{j}"))
                # -|x1|^2/2 keys-bias row, base partition 0 (PE operand)
                brow2 = px1T.tile([1, N], dt.float32r, tag="brow2", name="brow2")

                with (
                    tc.tile_pool(name="pld", bufs=2) as pld,
                    tc.tile_pool(name="pstb", bufs=2) as pstb,
                    tc.tile_pool(name="ptp", bufs=4, space="PSUM") as ptp,
                ):
                    ident = pld.tile([P, P], dt.float32, tag="ident")
                    masks.make_identity(nc, ident[:])
                    for t in range(N_TILES):
                        cs = slice(t * P, (t + 1) * P)
                        xx = pld.tile([P, XW], dt.float32, tag="xxl")
                        nc.sync.dma_start(xx[:], x1d[cs, :])
                        stb = pstb.tile([P, 7 * P], dt.bfloat16, tag="stb")
                        nc.gpsimd.memset(stb[96:128, 6 * P:7 * P], 0.0)
                        for j in range(7):
                            w = 128 if j < 6 else 97
                            kd = 128 if j < 6 else 96
                            ps = ptp.tile([P, P], dt.float32, tag="tps")
                            nc.tensor.matmul(
                                ps[:w, :], lhsT=xx[:, j * P:j * P + w],
                                rhs=ident[:], is_transpose=True,
                                start=True, stop=True, skip_group_check=True,
                            )
                            nc.scalar.activation(
                                x1T[j][:kd, cs], ps[:kd, :], AF.Copy
                            )
                            if j == 6:
                                nc.scalar.activation(
                                    brow2[:, cs], ps[96:97, :], AF.Copy
                                )
                            nc.vector.tensor_copy(
                                stb[:kd, j * P:(j + 1) * P], ps[:kd, :]
                            )
                        nc.sync.dma_start(x1bs[t], stb[:])

                tc.strict_bb_all_engine_barrier()

                # ================ PHASE C: block2 gram/rerank/y_a ============
                with (
                    tc.tile_pool(name="pk2", bufs=2) as pk2,
                    tc.tile_pool(name="pbs2", bufs=2) as pbs2,
                    tc.tile_pool(name="pbc2", bufs=1) as pbc2,
                    tc.tile_pool(name="pstr", bufs=2) as pstr,
                    tc.tile_pool(name="pac2", bufs=1) as pac2,
                    tc.tile_pool(name="pgq2", bufs=2, space="PSUM") as pgq2,
                    tc.tile_pool(name="py2", bufs=2, space="PSUM") as py2,
                ):
                    def y2a_tile(yt):
                        cs = slice(yt * P, (yt + 1) * P)
                        stq = pstr.tile([P, 7 * P], dt.bfloat16, tag="stq")
                        nc.sync.dma_start(stq[:], x1bs[yt])
                        sb = pac2.tile([P, O2], dt.bfloat16, tag="y2a_sb")
                        for hh in range(2):
                            ps = py2.tile([P, 864], dt.float32, tag="y2ps",
                                          name=f"y2ps{hh}")
                            for (oo, ow) in OS2[hh * 2:hh * 2 + 2]:
                                po = oo - hh * 864
                                for ci, (off, kk) in enumerate(KS2):
                                    nc.tensor.matmul(
                                        ps[:, po:po + ow],
                                        lhsT=stq[:kk, ci * P:(ci + 1) * P],
                                        rhs=w2a_t[ci][:, oo:oo + ow],
                                        start=(ci == 0), stop=(ci == len(KS2) - 1),
                                        skip_group_check=True,
                                    )
                            nc.scalar.activation(
                                sb[:, hh * 864:(hh + 1) * 864], ps[:], AF.Copy
                            )
                        nc.sync.dma_start(yad2[cs, :], sb[:])

                    for t in range(H_TILES):
                        cs = slice(t * P, (t + 1) * P)
                        keys = pk2.tile([P, N], dt.float32, tag="keys2")
                        for q in range(4):
                            ps = pgq2.tile([P, 1024], dt.float32, tag="gps2")
                            for si in range(2):
                                nsl = slice(q * 1024 + si * 512,
                                            q * 1024 + si * 512 + 512)
                                psl = slice(si * 512, si * 512 + 512)
                                for ci, (off, kk) in enumerate(KS2):
                                    kd = 128 if ci < 6 else 96
                                    nc.tensor.matmul(
                                        ps[:, psl],
                                        lhsT=x1T[ci][:kd, cs],
                                        rhs=x1T[ci][:kd, nsl],
                                        start=(ci == 0), stop=False,
                                        skip_group_check=True,
                                    )
                                nc.tensor.matmul(
                                    ps[:, psl],
                                    lhsT=onest_t[:],
                                    rhs=brow2[:, nsl],
                                    start=False, stop=True,
                                    skip_group_check=True,
                                )
                            nc.scalar.activation(
                                keys[:, q * 1024:(q + 1) * 1024], ps[:], AF.Copy
                            )
                        rerank(pbs2, pbc2, t, keys, x1d, XW, C2, pidx2, idx2_t)
                        y2a_tile(2 * t)
                        y2a_tile(2 * t + 1)

                px1T_cm.__exit__(None, None, None)

                # ================ PHASE D: block2 epilogue ===================
                with (
                    tc.tile_pool(name="pwc2", bufs=1) as pwc2,
                    tc.tile_pool(name="pstr2", bufs=3) as pstr2,
                    tc.tile_pool(name="pg5b", bufs=2) as pg5b,
                    tc.tile_pool(name="pe2b", bufs=3) as pe2b,
                    tc.tile_pool(name="pyc2", bufs=2, space="PSUM") as pyc2,
                ):
                    w2c_t = []
                    for ci, (off, kk) in enumerate(KS2):
                        tt = pwc2.tile([kk, O2], dt.bfloat16, tag=f"w2c{ci}")
                        nc.sync.dma_start(tt[:], w2cb[off:off + kk, :])
                        w2c_t.append(tt)
                    b2row = pwc2.tile([1, O2], dt.bfloat16, tag="b2row")
                    nc.sync.dma_start(b2row[:], w2cb[C2:C2 + 1, :])
                    tc.strict_bb_all_engine_barrier()
                    for t in range(H_TILES):
                        cs = slice(t * P, (t + 1) * P)
                        stq = pstr2.tile([P, 7 * P], dt.bfloat16, tag="stq2")
                        nc.sync.dma_start(stq[:], x1bs[t])
                        yct = pe2b.tile([P, O2], dt.float32, tag="yct2")
                        for hh in range(2):
                            ps = pyc2.tile([P, 864], dt.float32, tag="yc2ps",
                                           name=f"yc2ps{hh}")
                            for (oo, ow) in OS2[hh * 2:hh * 2 + 2]:
                                po = oo - hh * 864
                                for ci, (off, kk) in enumerate(KS2):
                                    nc.tensor.matmul(
                                        ps[:, po:po + ow],
                                        lhsT=stq[:kk, ci * P:(ci + 1) * P],
                                        rhs=w2c_t[ci][:, oo:oo + ow],
                                        start=(ci == 0), stop=False,
                                        skip_group_check=True,
                                    )
                                nc.tensor.matmul(
                                    ps[:, po:po + ow],
                                    lhsT=onesb_t[:],
                                    rhs=b2row[:, oo:oo + ow],
                                    start=False, stop=True,
                                    skip_group_check=True,
                                )
                            nc.scalar.activation(
                                yct[:, hh * 864:(hh + 1) * 864], ps[:], AF.Copy
                            )
                        g5 = pg5b.tile([P, K, O2], dt.bfloat16, tag="g5b")
                        nc.sync.dma_start(g5[:, 0, :], yad2[cs, :])
                        for j in range(K - 1):
                            nc.gpsimd.indirect_dma_start(
                                out=g5[:, j + 1, :],
                                out_offset=None,
                                in_=yad2[:],
                                in_offset=bass.IndirectOffsetOnAxis(
                                    ap=idx2_t[t][:, j:j + 1], axis=0
                                ),
                            )
                        mb = pe2b.tile([P, O2], dt.bfloat16, tag="mb2")
                        nc.vector.tensor_tensor(out=mb[:], in0=g5[:, 0, :],
                                                in1=g5[:, 1, :], op=OP.max)
                        for j in range(2, K):
                            nc.vector.tensor_tensor(out=mb[:], in0=mb[:],
                                                    in1=g5[:, j, :], op=OP.max)
                        xo = pe2b.tile([P, O2], dt.float32, tag="xo")
                        nc.vector.tensor_tensor(out=xo[:], in0=mb[:],
                                                in1=yct[:], op=OP.add)
                        nc.scalar.activation(xo[:], xo[:], AF.Prelu, alpha=SLOPE)
                        nc.sync.dma_start(xout[cs, :], xo[:])

            pidx2_cm.__exit__(None, None, None)
            pidx1_cm.__exit__(None, None, None)

    nc.finalize()
    return nc


_CACHE = {}


def _get_programs():
    if "p" not in _CACHE:
        _CACHE["p"] = _build_fused()
    return (_CACHE["p"],)


# ---------------------------------------------------------------- host side

def _fold_bn(W, gamma, beta, mean, var, cin):
    s = gamma.astype(np.float64) / np.sqrt(var.astype(np.float64) + EPS)
    Wp = s[:, None] * W.astype(np.float64)
    Wa = Wp[:, :cin].T
    Wc = (Wp[:, cin:] - Wp[:, :cin]).T
    bp = beta.astype(np.float64) - s * mean.astype(np.float64)
    return (np.ascontiguousarray(Wa, np.float32),
            np.ascontiguousarray(Wc, np.float32),
            bp.astype(np.float32))


def _prep_core(xp, W1a, W1c, b1, w2a_b, w2cb_b):
    xT = np.ascontiguousarray(xp.T)
    sq = np.einsum("nc,nc->n", xp.astype(np.float64), xp.astype(np.float64))
    bias_row = _tf32((-sq / 2).astype(np.float32))[None, :]
    xa = np.zeros((N, XA1), np.float32)
    xa[:, :C1] = xp
    xa[:, C1] = (-sq / 2).astype(np.float32)
    return dict(
        xTr=np.concatenate([_tf32(xT), bias_row], axis=0),
        xa1=xa,
        wa1=W1a, wc1=W1c,
        b1b=np.broadcast_to(b1, (P, O1)).copy(),
        w2a=w2a_b, w2cb=w2cb_b,
        onest=np.ones((1, P), np.float32),
        onesb=np.ones((1, P), ml_dtypes.bfloat16),
        io8=np.broadcast_to(np.arange(8, dtype=np.float32), (P, 8)).copy(),
    )


_LAST_EXEC_NS = {"l1": None}


def kernel(interm_repr, W1, bn1_gamma, bn1_beta, bn1_mean, bn1_var,
           W2, bn2_gamma, bn2_beta, bn2_mean, bn2_var, _trace=False):
    x = np.asarray(interm_repr, dtype=np.float32)
    (p,) = _get_programs()

    W1a, W1c, b1 = _fold_bn(np.asarray(W1), np.asarray(bn1_gamma),
                            np.asarray(bn1_beta), np.asarray(bn1_mean),
                            np.asarray(bn1_var), C1)
    W2a, W2c, b2 = _fold_bn(np.asarray(W2), np.asarray(bn2_gamma),
                            np.asarray(bn2_beta), np.asarray(bn2_mean),
                            np.asarray(bn2_var), C2)
    W1a = _tf32(W1a)
    W1c = _tf32(W1c)
    w2a_b = _bf16(W2a)
    w2cb_b = _bf16(np.concatenate([W2c, b2[None, :]], axis=0))

    in_maps = []
    for c in range(8):
        b, h = c // 2, c % 2
        perm = np.r_[h * HALF:(h + 1) * HALF, (1 - h) * HALF:(2 - h) * HALF]
        in_maps.append(_prep_core(x[b][perm], W1a, W1c, b1, w2a_b, w2cb_b))
    r = run_bass_kernel_spmd(p, in_maps, core_ids=list(range(8)), trace=_trace)
    _LAST_EXEC_NS["l1"] = r.exec_time_ns

    x2 = np.empty((B, N, O2), np.float32)
    for c in range(8):
        b, h = c // 2, c % 2
        x2[b, h * HALF:(h + 1) * HALF] = r.results[c]["xout"]
    return x2


if __name__ == "__main__":
    rng = np.random.default_rng(0)
    inp = dict(
        interm_repr=rng.standard_normal((B, N, C1), dtype=np.float32),
        W1=(rng.standard_normal((O1, 2 * C1)) / np.sqrt(2 * C1)).astype(np.float32),
        bn1_gamma=1 + 0.1 * rng.standard_normal(O1).astype(np.float32),
        bn1_beta=0.1 * rng.standard_normal(O1).astype(np.float32),
        bn1_mean=0.1 * rng.standard_normal(O1).astype(np.float32),
        bn1_var=0.5 + rng.random(O1).astype(np.float32),
        W2=(rng.standard_normal((O2, 2 * C2)) / np.sqrt(2 * C2)).astype(np.float32),
        bn2_gamma=1 + 0.1 * rng.standard_normal(O2).astype(np.float32),
        bn2_beta=0.1 * rng.standard_normal(O2).astype(np.float32),
        bn2_mean=0.1 * rng.standard_normal(O2).astype(np.float32),
        bn2_var=0.5 + rng.random(O2).astype(np.float32),
    )
    out = kernel(**inp)
    print("kernel out", out.shape, out.dtype, np.abs(out).mean())
